# revision 16
# baseline (speedup 1.0000x reference)
"""Trainium2 Bass kernel for causal multi-head attention with RoPE
(nn_Attention: S=2048, D=4096, H=32, hd=128), tensor-parallel over heads
across 8 NeuronCores.

v2 strategy (per core, 4 heads):
  - Q^T/K^T projections head-major in [hd, S] layout (lhsT = W tile,
    rhs = x^T strip), bf16. RoPE via host-permuted [re;im] split:
    rot = raw*C2 + swap(raw)*S2m with a 128x128 swap matmul on the PE.
  - V projected DIRECTLY into natural [t, hd] layout: lhsT = x^T block
    [k,t-128] (stationary), rhs = Wv columns of all 4 heads [k, 512].
    No PE transposes for V; Wv is persistent in SBUF (loaded once).
  - Attention computes scores TRANSPOSED: scoresT[t, s-chunk] =
    (K^T tile)^T @ Q^T, so exp(scoresT) on ScalarE lands directly in the
    P^T layout that the PV matmul streams -- the per-block PE transposes
    of P from v1 are gone entirely.  Causal masking: t-tiles past the
    diagonal are skipped (ragged s_lo starts); the diagonal 128-block is
    masked multiplicatively (triu) on the DVE after exp.
  - Softmax denominators: rowsum over t is a partition-axis sum, done as
    one extra PE pass per head with an all-ones 128x128 stationary
    (ldweights deduped): out[p, s] = sum_t P^T[t, s] for every p, i.e.
    the rowsum is produced pre-broadcast across all partitions.  A
    single reciprocal_approx_fast (DVE) gives 1/rowsum, and the
    normalize is fused into the psum->sbuf copy of A^T (tensor_mul).
  - Output projection unchanged: O^T partial accumulated over the 4
    local heads, 4 concurrent psum groups sharing the stationary.
    Partials are written bf16; host sums the 8 partials in float64.

Scheduling: weight/const DMAs issue from the (otherwise idle) GpSimd
queue, x^T strips + outputs from SP.  ScalarE runs ONLY exp.  Emission
interleaves chunk ch's attention with chunk ch+1's q/k projections for
heads 0-1 and chunk ch's output projection with the remaining
projection units, so the PE stays fed through the Act-heavy late-chunk
attention windows.  x^T strips prefetch two windows ahead.
"""

import math
import sys
import types

import numpy as np
import ml_dtypes

import concourse.bass as bass
import concourse.tile as tile
import concourse.mybir as mybir
from concourse import bass_utils

BF16 = mybir.dt.bfloat16
F32 = mybir.dt.float32
P = 128


def enable_ldw_opt():
    """Flip walrus's --enable-ldw-opt to true (bass_utils hardcodes false).
    Patches run_command to rewrite the flag in the walrus argv."""
    import os
    if os.environ.get("BASS_LDW_OPT", "0") != "1":
        return
    if getattr(bass_utils, "_ldw_patch", False):
        return
    orig = bass_utils.run_command

    def patched(argv, **kwargs):
        argv = ["--enable-ldw-opt=true" if a == "--enable-ldw-opt=false" else a
                for a in argv]
        return orig(argv, **kwargs)

    bass_utils.run_command = patched
    bass_utils._ldw_patch = True


def install_ntff_hook_shim():
    """Make trace=True work under axon (antenv.axon_hooks is absent here)."""
    try:
        import antenv.axon_hooks  # noqa
        return
    except ImportError:
        pass
    try:
        import antenv
        from trn_agent_boot.trn_boot import _ntff_profile_via_ctypes
        hook = _ntff_profile_via_ctypes('/opt/axon/libaxon_pjrt.so')
        mod = types.ModuleType('antenv.axon_hooks')
        mod.get_axon_ntff_profile_hook = lambda: hook
        mod.set_axon_ntff_profile_hook = lambda h: None
        sys.modules['antenv.axon_hooks'] = mod
        antenv.axon_hooks = mod
    except Exception:
        pass


def dedup_ldweights(nc):
    """Remove an InstLdweights when the immediately preceding PE weight load
    has an identical stationary operand (consecutive matmuls sharing lhsT).
    Any waits on the removed load are transferred to the next instruction."""
    import concourse.mybir as _mb
    n = 0
    for f in nc.m.functions:
        for bb in f.blocks:
            new = []
            last_key = None
            pending_waits = []
            for inst in bb.instructions:
                ty = type(inst).__name__
                eng = getattr(inst, "engine", None)
                if eng == _mb.EngineType.PE:
                    if ty == "InstLdweights":
                        o = inst.ins[0]
                        key = (str(getattr(o, "memref", "")), o.offset,
                               str(o.ap), str(getattr(o, "dtype", "")),
                               getattr(inst, "is_transpose", None),
                               getattr(inst, "tile_position", None))
                        if key == last_key:
                            si = getattr(inst, "sync_info", None)
                            if si is not None and si.on_wait:
                                pending_waits.extend(si.on_wait)
                            n += 1
                            continue   # drop this load
                        last_key = key
                    elif ty in ("InstMatmult", "InstEventSemaphore", "InstNoOp"):
                        pass           # none of these clobber loaded weights
                    else:
                        last_key = None
                    if pending_waits:
                        si = getattr(inst, "sync_info", None)
                        if si is None:
                            inst.sync_info = _mb.SyncInfo(
                                on_wait=list(pending_waits), on_update=[])
                        else:
                            si.on_wait = list(pending_waits) + list(si.on_wait)
                        pending_waits = []
                new.append(inst)
            assert not pending_waits
            bb.instructions[:] = new
    return n


def split_excess_waits(nc, max_waits=1):
    """This walrus build accepts only one sync-wait per instruction; split
    extra waits into preceding wait-only NoOps on the same engine."""
    n = 0
    for f in nc.m.functions:
        for bb in f.blocks:
            new = []
            for inst in bb.instructions:
                si = getattr(inst, "sync_info", None)
                waits = list(si.on_wait) if (si is not None and si.on_wait) else []
                if len(waits) > max_waits:
                    extra, keep = waits[:-max_waits], waits[-max_waits:]
                    for j, w in enumerate(extra):
                        new.append(mybir.InstNoOp(
                            name=f"{inst.name}_sw{j}",
                            engine=inst.engine,
                            bass_nofuse=True,
                            sync_info=mybir.SyncInfo(on_wait=[w], on_update=[]),
                        ))
                    si.on_wait = keep
                    n += 1
                new.append(inst)
            bb.instructions[:] = new
    return n


class Cfg:
    def __init__(self, S=2048, D=4096, H_LOC=4, CHUNK=512, n_cores=8):
        self.S = S              # sequence length
        self.D = D              # model dim (= contraction dim of projections)
        self.H_LOC = H_LOC      # heads per core
        self.CHUNK = CHUNK      # s-chunk size (outer loop granularity)
        self.n_cores = n_cores
        self.NK = D // P        # k-tiles in projections
        self.NCH = S // CHUNK   # number of s-chunks
        self.TPC = CHUNK // P   # s/t tiles per chunk (must be 4 for 512)
        self.DLOC = H_LOC * P   # local head dims
        self.SCALE = 1.0 / math.sqrt(P)  # 1/sqrt(hd)


FULL = Cfg()


def build_program(cfg: Cfg):
    """Builds the per-core Bass/Tile program (SPMD: same NEFF on all cores)."""
    S, NK, H_LOC, CHUNK, NCH, TPC = cfg.S, cfg.NK, cfg.H_LOC, cfg.CHUNK, cfg.NCH, cfg.TPC
    DLOC = cfg.DLOC

    nc = bass.Bass("TRN2", target_bir_lowering=False, debug=False,
                   num_devices=cfg.n_cores)

    # ---- DRAM I/O ----
    xt_d = nc.dram_tensor("xt", [NCH, 2, P, (NK // 2) * CHUNK], BF16,
                          kind="ExternalInput").ap()
    wq_d = nc.dram_tensor("wq", [H_LOC, P, NK * P], BF16, kind="ExternalInput").ap()
    wk_d = nc.dram_tensor("wk", [H_LOC, P, NK * P], BF16, kind="ExternalInput").ap()
    wv_d = nc.dram_tensor("wv", [P, NK, DLOC], BF16, kind="ExternalInput").ap()
    wo_d = nc.dram_tensor("wo", [cfg.D // CHUNK, P, H_LOC * CHUNK], BF16,
                          kind="ExternalInput").ap()
    cos_d = nc.dram_tensor("cosS", [P, S], BF16, kind="ExternalInput").ap()
    sin_d = nc.dram_tensor("sinm", [P, S], BF16, kind="ExternalInput").ap()
    triu_d = nc.dram_tensor("triu", [P, P], BF16, kind="ExternalInput").ap()
    ones_d = nc.dram_tensor("ones128", [P, P], BF16, kind="ExternalInput").ap()
    swp_d = nc.dram_tensor("swap128", [P, P], BF16, kind="ExternalInput").ap()
    ot_d = nc.dram_tensor("ot", [S, cfg.D], BF16, kind="ExternalOutput").ap()

    with tile.TileContext(nc) as tc:
        with tc.tile_pool(name="const", bufs=1) as const_pool, \
             tc.tile_pool(name="persist", bufs=1) as persist, \
             tc.tile_pool(name="xtp", bufs=2) as xtp, \
             tc.tile_pool(name="wqk", bufs=2) as wqkp, \
             tc.tile_pool(name="qtp", bufs=H_LOC + 3) as qtp, \
             tc.tile_pool(name="rawp", bufs=4) as rawp, \
             tc.tile_pool(name="pp", bufs=4 * TPC + 4) as pp, \
             tc.tile_pool(name="atp", bufs=H_LOC + 2) as atp, \
             tc.tile_pool(name="recp", bufs=2) as recp, \
             tc.tile_pool(name="osbp", bufs=4) as osbp, \
             tc.tile_pool(name="psA", bufs=2, space="PSUM") as psA, \
             tc.tile_pool(name="psS", bufs=2, space="PSUM") as psS, \
             tc.tile_pool(name="psAT", bufs=3, space="PSUM") as psAT, \
             tc.tile_pool(name="psR", bufs=1, space="PSUM") as psR:

            # constants (gpsimd DMA queue; small transfers, emitted after the
            # first W pieces so they don't delay the first matmul)
            triu = const_pool.tile([P, P], BF16, name="triu")
            ones128 = const_pool.tile([P, P], BF16, name="ones128")
            swap128 = const_pool.tile([P, P], BF16, name="swap128")
            cosS = const_pool.tile([P, S], BF16, name="cosS")
            sinm = const_pool.tile([P, S], BF16, name="sinm")

            def emit_cos_sin():
                nc.gpsimd.dma_start(swap128, swp_d)
                for j in range(4):
                    sl = slice(j * (S // 4), (j + 1) * (S // 4))
                    nc.gpsimd.dma_start(cosS[:, sl], cos_d[:, sl])
                    nc.gpsimd.dma_start(sinm[:, sl], sin_d[:, sl])
                nc.gpsimd.dma_start(triu, triu_d)
                nc.gpsimd.dma_start(ones128, ones_d)

            # persistent tensors: K^T per head, natural V, Wv, Wo
            KT = []
            for h in range(H_LOC):
                kt_h = persist.tile([P, S], BF16, name=f"kt{h}", tag=f"kt{h}")
                KT.append(kt_h)
            Vn = persist.tile([P, S // P, DLOC], BF16, name="vnat", tag="vnat")
            wv_pers = persist.tile([P, NK, DLOC], BF16, name="wv_pers",
                                   tag="wv_pers")
            NGR = cfg.D // CHUNK
            wo_pers = persist.tile([P, NGR, H_LOC, CHUNK], BF16,
                                   name="wo_pers", tag="wo_pers")

            def emit_wv():
                for q in range(8):
                    ksl = slice(q * (NK // 8), (q + 1) * (NK // 8))
                    nc.sync.dma_start(wv_pers[:, ksl, :], wv_d[:, ksl, :])

            NKH = NK // 2
            xts_all = {}     # ch -> [half0, half1]
            qt_all = {}      # (ch, h) -> qt tile
            at_all = {}      # (ch, h) -> at tile

            def alloc_xt(ch):
                xts = [xtp.tile([P, NKH, CHUNK], BF16,
                                name=f"xt_{ch}_{half}", tag="xt")
                       for half in range(2)]
                xts_all[ch] = xts

            def emit_xt_quarter(ch, q8, fine=False):
                """One of 8 quarter-DMAs for chunk ch's x^T strip."""
                half, q = divmod(q8, 4)
                xh = xts_all[ch][half]
                src = xt_d[ch, half].rearrange("p (k c) -> p k c", c=CHUNK)
                kq = NKH // 4
                if fine:
                    for j in range(kq):
                        ksl = slice(q * kq + j, q * kq + j + 1)
                        nc.sync.dma_start(xh[:, ksl, :], src[:, ksl, :])
                else:
                    ksl = slice(q * kq, (q + 1) * kq)
                    nc.sync.dma_start(xh[:, ksl, :], src[:, ksl, :])

            def emit_xt(ch, fine=False):
                alloc_xt(ch)
                for q8 in range(8):
                    emit_xt_quarter(ch, q8, fine=(fine and q8 == 0))

            def emit_qk_unit(ch, which, h, after_w_hook=None):
                """One q/k projection unit: W load + NK matmuls + RoPE."""
                s0 = ch * CHUNK
                xts = xts_all[ch]
                w_dram = {"q": wq_d, "k": wk_d}[which]
                wt = wqkp.tile([P, NK, P], BF16,
                               name=f"w{which}_{ch}_{h}", tag="wqk")
                wsrc = w_dram[h].rearrange("p (k m) -> p k m", m=P)
                npieces = (8 if which == "q" else 4) if (ch == 0 and h == 0) else 2
                for q in range(npieces):
                    ksl = slice(q * (NK // npieces), (q + 1) * (NK // npieces))
                    nc.gpsimd.dma_start(wt[:, ksl, :], wsrc[:, ksl, :])
                if after_w_hook is not None:
                    after_w_hook()
                ps = psA.tile([P, CHUNK], F32,
                              name=f"ps_{which}_{ch}_{h}", tag="psA")
                for k in range(NK):
                    nc.tensor.matmul(ps, wt[:, k, :],
                                     xts[k // NKH][:, k % NKH, :],
                                     start=(k == 0), stop=(k == NK - 1))
                raw = rawp.tile([P, CHUNK], BF16,
                                name=f"raw_{which}_{ch}_{h}", tag="raw")
                nc.vector.tensor_copy(raw, ps)
                # RoPE: rot = raw*C2 + swap(raw)*S2m
                ps2 = psR.tile([P, CHUNK], F32,
                               name=f"psw_{which}_{ch}_{h}", tag="psR")
                nc.tensor.matmul(ps2, swap128, raw, start=True, stop=True)
                if which == "q":
                    dst = qtp.tile([P, CHUNK], BF16,
                                   name=f"qt_{ch}_{h}", tag="qt")
                    qt_all[(ch, h)] = dst
                else:
                    dst = KT[h][:, s0:s0 + CHUNK]
                tmp2 = rawp.tile([P, CHUNK], BF16,
                                 name=f"tmp2_{which}_{ch}_{h}", tag="tmp2")
                nc.vector.tensor_mul(dst, raw, cosS[:, s0:s0 + CHUNK])
                nc.vector.tensor_mul(tmp2, ps2, sinm[:, s0:s0 + CHUNK])
                nc.vector.tensor_add(dst, dst, tmp2)

            def emit_v_unit(ch, tl):
                """V projection for one t-tile, all heads, directly in natural
                [t, hd] layout: stationary = x^T block, moving = Wv columns."""
                xts = xts_all[ch]
                ps = psA.tile([P, DLOC], F32, name=f"psv_{ch}_{tl}", tag="psA")
                tsl = slice(tl * P, (tl + 1) * P)
                for k in range(NK):
                    nc.tensor.matmul(ps, xts[k // NKH][:, k % NKH, tsl],
                                     wv_pers[:, k, :],
                                     start=(k == 0), stop=(k == NK - 1))
                # V units run in outproj windows where ScalarE is idle
                nc.scalar.copy(Vn[:, ch * TPC + tl, :], ps)

            def emit_attn_head(ch, h):
                """Attention for (chunk, head): transposed scores -> exp ->
                PV, rowsum via ones-stationary pass, fused normalize."""
                n_t = (ch + 1) * TPC
                qt_h = qt_all[(ch, h)]
                psat = psAT.tile([P, CHUNK], F32, name=f"psat_{ch}_{h}",
                                 tag="psAT")
                pts = []
                pending = None
                for tb in range(n_t):
                    s_lo = max(0, tb - ch * TPC) * P
                    pss = psS.tile([P, CHUNK], F32,
                                   name=f"pss_{ch}_{h}_{tb}", tag="psS")
                    nc.tensor.matmul(pss[:, s_lo:], KT[h][:, tb * P:(tb + 1) * P],
                                     qt_h[:, s_lo:], start=True, stop=True)
                    pt = pp.tile([P, CHUNK], BF16,
                                 name=f"pt_{ch}_{h}_{tb}", tag="pt")
                    nc.scalar.activation(pt[:, s_lo:], pss[:, s_lo:],
                                         mybir.ActivationFunctionType.Exp,
                                         scale=cfg.SCALE)
                    if tb >= ch * TPC:
                        nc.vector.tensor_mul(pt[:, s_lo:s_lo + P],
                                             pt[:, s_lo:s_lo + P], triu)
                    if pending is not None:
                        ptb, plo, ppt = pending
                        nc.tensor.matmul(psat[:, plo:],
                                         Vn[:, ptb, h * P:(h + 1) * P],
                                         ppt[:, plo:],
                                         start=(ptb == 0), stop=False)
                    pending = (tb, s_lo, pt)
                    pts.append((tb, s_lo, pt))
                ptb, plo, ppt = pending
                nc.tensor.matmul(psat[:, plo:], Vn[:, ptb, h * P:(h + 1) * P],
                                 ppt[:, plo:], start=(ptb == 0), stop=True)
                # rowsum over t (partition axis) via all-ones stationary:
                # every output partition receives sum_t P^T[t, s] -- i.e. the
                # rowsum arrives pre-broadcast.  Consecutive matmuls share the
                # ones stationary (deduped to one ldweights).
                rs = psR.tile([P, CHUNK], F32, name=f"rs_{ch}_{h}", tag="psR")
                for tb, s_lo, pt in pts:
                    nc.tensor.matmul(rs[:, s_lo:], ones128, pt[:, s_lo:],
                                     start=(tb == 0), stop=(tb == n_t - 1))
                # 1/rowsum as exp(-ln(rowsum)) on ScalarE: both functions live
                # in the natural_log_exp table set (no table switching), and
                # the DVE reciprocal at [128,512] would cost 8 cyc/element.
                lnrs = recp.tile([P, CHUNK], F32,
                                 name=f"lnrs_{ch}_{h}", tag="lnrs")
                nc.scalar.activation(lnrs, rs,
                                     mybir.ActivationFunctionType.Ln)
                recipb = recp.tile([P, CHUNK], F32,
                                   name=f"rec_{ch}_{h}", tag="rec")
                nc.scalar.activation(recipb, lnrs,
                                     mybir.ActivationFunctionType.Exp,
                                     scale=-1.0)
                at_h = atp.tile([P, CHUNK], BF16, name=f"at_{ch}_{h}", tag="at")
                nc.vector.tensor_mul(at_h, psat, recipb)
                at_all[(ch, h)] = at_h

            def emit_outproj(ch, filler_units):
                """Output projection for s-chunk ch, interleaved with the
                given list of zero-arg emit callbacks (projection units /
                prefetches) so the PE never starves."""
                s0 = ch * CHUNK
                at_cur = [at_all[(ch, h)] for h in range(H_LOC)]
                nu = len(filler_units)
                NBLK = (NGR + 3) // 4
                n_iters = TPC * NBLK
                for it in range(n_iters):
                    stl, blk = divmod(it, NBLK)
                    ngs = list(range(blk * 4, min(blk * 4 + 4, NGR)))
                    psos = []
                    for j in range(len(ngs)):
                        pool, tg = (psS, "psS") if j < 2 else (psAT, "psAT")
                        pso = pool.tile([P, CHUNK], F32,
                                        name=f"pso_{ch}_{stl}_{blk}_{j}", tag=tg)
                        psos.append(pso)
                    for h in range(H_LOC):
                        lhs = at_cur[h][:, stl * P:(stl + 1) * P]
                        for j, ng in enumerate(ngs):
                            nc.tensor.matmul(psos[j], lhs, wo_pers[:, ng, h, :],
                                             start=(h == 0),
                                             stop=(h == H_LOC - 1))
                    for j, ng in enumerate(ngs):
                        osb = osbp.tile([P, CHUNK], BF16,
                                        name=f"osb_{ch}_{stl}_{blk}_{j}", tag="osb")
                        if j % 2:
                            nc.scalar.copy(osb, psos[j])
                        else:
                            nc.vector.tensor_copy(osb, psos[j])
                        srow = s0 + stl * P
                        if ch == NCH - 1 and it == n_iters - 1 and j % 2:
                            eng = nc.gpsimd
                        else:
                            eng = nc.sync
                        eng.dma_start(
                            ot_d[srow:srow + P, ng * CHUNK:(ng + 1) * CHUNK], osb)
                    for u in range(it * nu // n_iters,
                                   (it + 1) * nu // n_iters):
                        filler_units[u]()

            # ---------------- emission schedule ----------------
            emit_xt(0, fine=True)
            first = [True]

            def _cos_hook():
                if first[0]:
                    emit_cos_sin()
                    first[0] = False

            # chunk 0 projections, all up front.  GpSimd DMA queue order:
            # consts, Wq0+cos/sin, Wk0, Wv (4MB), remaining W, Wo -- each
            # lands just before its first consumer.  x^T(1) prefetches on the
            # SP queue behind x^T(0).
            emit_qk_unit(0, "q", 0, after_w_hook=_cos_hook)
            emit_qk_unit(0, "k", 0)
            emit_wv()
            if NCH > 1:
                emit_xt(1)
            for tl in range(TPC):
                emit_v_unit(0, tl)
            for h in range(1, H_LOC):
                emit_qk_unit(0, "q", h)
                emit_qk_unit(0, "k", h)

            def emit_wo():
                for ng in range(NGR):
                    nc.gpsimd.dma_start(
                        wo_pers[:, ng],
                        wo_d[ng].rearrange("p (h c) -> p h c", c=CHUNK))

            for ch in range(NCH):
                nxt = ch + 1
                # ---- attention window: heads of ch, interleaved with q/k
                # units of chunk ch+1 for heads 0..1 and x^T prefetch ----
                if 2 <= nxt < NCH:
                    # x^T(ch+1) prefetch; its buffers (chunk ch-1's) are
                    # long free -- proj(ch) finished last window
                    emit_xt(nxt)
                for h in range(H_LOC):
                    emit_attn_head(ch, h)
                    if nxt < NCH and h < 2:
                        emit_qk_unit(nxt, "q", h)
                        emit_qk_unit(nxt, "k", h)
                    if ch == 0 and h == min(1, H_LOC - 1):
                        # Wo load (4MB) behind the attn-window W units but
                        # well before outproj(0) needs it
                        emit_wo()
                # ---- output projection window: interleave v(ch+1) units and
                # the remaining q/k units of ch+1 ----
                fillers = []
                if nxt < NCH:
                    for tl in range(TPC):
                        fillers.append(lambda tl=tl: emit_v_unit(nxt, tl))
                    for h in range(2, H_LOC):
                        fillers.append(lambda h=h: emit_qk_unit(nxt, "q", h))
                        fillers.append(lambda h=h: emit_qk_unit(nxt, "k", h))
                emit_outproj(ch, fillers)

    dedup_ldweights(nc)
    split_excess_waits(nc)
    return nc


# ---------------- host-side data prep ----------------

def _tile_w(w_cols: np.ndarray, NK: int) -> np.ndarray:
    """[D, 128] per-head weight slice -> [128, NK*128] (k-part, k-outer*col)."""
    D = w_cols.shape[0]
    return np.ascontiguousarray(
        w_cols.reshape(NK, P, P).transpose(1, 0, 2).reshape(P, NK * P))


_ROPE_PERM = np.concatenate([np.arange(0, P, 2), np.arange(1, P, 2)])


def prepare_core_inputs(cfg: Cfg, core: int, x, wq, wk, wv, wo, cos, sin):
    """Builds the in_map (dict of numpy arrays) for one core."""
    bf = ml_dtypes.bfloat16
    S, D, H_LOC, CHUNK, NK, NCH = cfg.S, cfg.D, cfg.H_LOC, cfg.CHUNK, cfg.NK, cfg.NCH
    DLOC = cfg.DLOC
    c0 = core * DLOC

    out = {}
    # xt: [NCH, 2, 128, (NK//2)*CHUNK]
    xt = np.empty((NCH, 2, P, (NK // 2) * CHUNK), dtype=bf)
    xTb = x.T.astype(bf)  # [D, S]
    for ch in range(NCH):
        for half in range(2):
            blk = xTb[half * (D // 2):(half + 1) * (D // 2),
                      ch * CHUNK:(ch + 1) * CHUNK]          # [D/2, CHUNK]
            blk = blk.reshape(NK // 2, P, CHUNK).transpose(1, 0, 2)
            xt[ch, half] = blk.reshape(P, (NK // 2) * CHUNK)
    out["xt"] = xt

    for name, w in (("wq", wq), ("wk", wk)):
        wt = np.empty((H_LOC, P, NK * P), dtype=bf)
        for h in range(H_LOC):
            cols = w[:, c0 + h * P: c0 + (h + 1) * P][:, _ROPE_PERM]
            wt[h] = _tile_w(cols.astype(bf), NK)
        out[name] = wt

    # wv: [128, NK, DLOC]; wv_t[p, k, j] = wv[k*128+p, c0+j]
    wv_loc = wv[:, c0:c0 + DLOC].astype(bf)                  # [D, DLOC]
    out["wv"] = np.ascontiguousarray(
        wv_loc.reshape(NK, P, DLOC).transpose(1, 0, 2))

    # wo: [D//CHUNK, 128, H_LOC*CHUNK]; wo[ng, p, h*CHUNK+nl] = Wo[c0+h*128+p, ng*CHUNK+nl]
    wo_loc = wo[c0:c0 + DLOC, :].astype(bf)  # [DLOC, D]
    wo_t = np.empty((D // CHUNK, P, H_LOC * CHUNK), dtype=bf)
    for ng in range(D // CHUNK):
        blk = wo_loc[:, ng * CHUNK:(ng + 1) * CHUNK]     # [DLOC, CHUNK]
        blk = blk.reshape(H_LOC, P, CHUNK).transpose(1, 0, 2)
        wo_t[ng] = blk.reshape(P, H_LOC * CHUNK)
    out["wo"] = wo_t

    cosT = cos.T.astype(np.float32)    # [64, S]
    sinT = sin.T.astype(np.float32)
    out["cosS"] = np.concatenate([cosT, cosT], 0).astype(bf)
    out["sinm"] = np.concatenate([-sinT, sinT], 0).astype(bf)

    # triu (incl. diagonal) masks the diagonal 128-block of P^T [t, s]
    out["triu"] = np.triu(np.ones((P, P), np.float32)).astype(bf)
    out["ones128"] = np.ones((P, P), np.float32).astype(bf)
    sw = np.zeros((P, P), np.float32)
    sw[(np.arange(P) + 64) % P, np.arange(P)] = 1.0
    out["swap128"] = sw.astype(bf)
    return out


_PROGRAM_CACHE = {}


def get_program(cfg: Cfg):
    key = (cfg.S, cfg.D, cfg.H_LOC, cfg.CHUNK, cfg.n_cores)
    if key not in _PROGRAM_CACHE:
        _PROGRAM_CACHE[key] = build_program(cfg)
    return _PROGRAM_CACHE[key]


def run(cfg: Cfg, inputs: dict, trace: bool = False):
    """Run the sharded kernel; returns (list of per-core ot partials, results obj)."""
    install_ntff_hook_shim()
    enable_ldw_opt()
    x = np.asarray(inputs["x"], np.float32)
    wq = np.asarray(inputs["weight_q"], np.float32)
    wk = np.asarray(inputs["weight_k"], np.float32)
    wv = np.asarray(inputs["weight_v"], np.float32)
    wo = np.asarray(inputs["weight_o"], np.float32)
    cos = np.asarray(inputs["freqs_cos"], np.float32)
    sin = np.asarray(inputs["freqs_sin"], np.float32)

    nc = get_program(cfg)
    in_maps = [prepare_core_inputs(cfg, c, x, wq, wk, wv, wo, cos, sin)
               for c in range(cfg.n_cores)]
    res = bass_utils.run_bass_kernel_spmd(
        nc, in_maps, core_ids=list(range(cfg.n_cores)), trace=trace)
    return [r["ot"] for r in res.results], res


def kernel(**inputs) -> np.ndarray:
    ots, _ = run(FULL, inputs, trace=False)
    acc = np.zeros(ots[0].shape, dtype=np.float64)
    for ot in ots:
        acc += np.asarray(ot, dtype=np.float64)
    return np.ascontiguousarray(acc.astype(np.float32))


# revision 17
# speedup vs baseline: 1.0244x; 1.0244x over previous
"""Trainium2 Bass kernel for causal multi-head attention with RoPE
(nn_Attention: S=2048, D=4096, H=32, hd=128), tensor-parallel over heads
across 8 NeuronCores.

v2 strategy (per core, 4 heads):
  - Q^T/K^T projections head-major in [hd, S] layout (lhsT = W tile,
    rhs = x^T strip), bf16. RoPE via host-permuted [re;im] split:
    rot = raw*C2 + swap(raw)*S2m with a 128x128 swap matmul on the PE.
  - V projected DIRECTLY into natural [t, hd] layout: lhsT = x^T block
    [k,t-128] (stationary), rhs = Wv columns of all 4 heads [k, 512].
    No PE transposes for V; Wv is persistent in SBUF (loaded once).
  - Attention computes scores TRANSPOSED: scoresT[t, s-chunk] =
    (K^T tile)^T @ Q^T, so exp(scoresT) on ScalarE lands directly in the
    P^T layout that the PV matmul streams -- the per-block PE transposes
    of P from v1 are gone entirely.  Causal masking: t-tiles past the
    diagonal are skipped (ragged s_lo starts); the diagonal 128-block is
    masked multiplicatively (triu) on the DVE after exp.
  - Softmax denominators: rowsum over t is a partition-axis sum, done as
    one extra PE pass per head with an all-ones 128x128 stationary
    (ldweights deduped): out[p, s] = sum_t P^T[t, s] for every p, i.e.
    the rowsum is produced pre-broadcast across all partitions.  A
    single reciprocal_approx_fast (DVE) gives 1/rowsum, and the
    normalize is fused into the psum->sbuf copy of A^T (tensor_mul).
  - Output projection unchanged: O^T partial accumulated over the 4
    local heads, 4 concurrent psum groups sharing the stationary.
    Partials are written bf16; host sums the 8 partials in float64.

Scheduling: weight/const DMAs issue from the (otherwise idle) GpSimd
queue, x^T strips + outputs from SP.  ScalarE runs ONLY exp.  Emission
interleaves chunk ch's attention with chunk ch+1's q/k projections for
heads 0-1 and chunk ch's output projection with the remaining
projection units, so the PE stays fed through the Act-heavy late-chunk
attention windows.  x^T strips prefetch two windows ahead.
"""

import math
import sys
import types

import numpy as np
import ml_dtypes

import concourse.bass as bass
import concourse.tile as tile
import concourse.mybir as mybir
from concourse import bass_utils

BF16 = mybir.dt.bfloat16
F32 = mybir.dt.float32
P = 128


def enable_ldw_opt():
    """Flip walrus's --enable-ldw-opt to true (bass_utils hardcodes false).
    Patches run_command to rewrite the flag in the walrus argv."""
    import os
    if os.environ.get("BASS_LDW_OPT", "0") != "1":
        return
    if getattr(bass_utils, "_ldw_patch", False):
        return
    orig = bass_utils.run_command

    def patched(argv, **kwargs):
        argv = ["--enable-ldw-opt=true" if a == "--enable-ldw-opt=false" else a
                for a in argv]
        return orig(argv, **kwargs)

    bass_utils.run_command = patched
    bass_utils._ldw_patch = True


def install_ntff_hook_shim():
    """Make trace=True work under axon (antenv.axon_hooks is absent here)."""
    try:
        import antenv.axon_hooks  # noqa
        return
    except ImportError:
        pass
    try:
        import antenv
        from trn_agent_boot.trn_boot import _ntff_profile_via_ctypes
        hook = _ntff_profile_via_ctypes('/opt/axon/libaxon_pjrt.so')
        mod = types.ModuleType('antenv.axon_hooks')
        mod.get_axon_ntff_profile_hook = lambda: hook
        mod.set_axon_ntff_profile_hook = lambda h: None
        sys.modules['antenv.axon_hooks'] = mod
        antenv.axon_hooks = mod
    except Exception:
        pass


def dedup_ldweights(nc):
    """Remove an InstLdweights when the immediately preceding PE weight load
    has an identical stationary operand (consecutive matmuls sharing lhsT).
    Any waits on the removed load are transferred to the next instruction."""
    import concourse.mybir as _mb
    n = 0
    for f in nc.m.functions:
        for bb in f.blocks:
            new = []
            last_key = None
            pending_waits = []
            for inst in bb.instructions:
                ty = type(inst).__name__
                eng = getattr(inst, "engine", None)
                if eng == _mb.EngineType.PE:
                    if ty == "InstLdweights":
                        o = inst.ins[0]
                        key = (str(getattr(o, "memref", "")), o.offset,
                               str(o.ap), str(getattr(o, "dtype", "")),
                               getattr(inst, "is_transpose", None),
                               getattr(inst, "tile_position", None))
                        if key == last_key:
                            si = getattr(inst, "sync_info", None)
                            if si is not None and si.on_wait:
                                pending_waits.extend(si.on_wait)
                            n += 1
                            continue   # drop this load
                        last_key = key
                    elif ty in ("InstMatmult", "InstEventSemaphore", "InstNoOp"):
                        pass           # none of these clobber loaded weights
                    else:
                        last_key = None
                    if pending_waits:
                        si = getattr(inst, "sync_info", None)
                        if si is None:
                            inst.sync_info = _mb.SyncInfo(
                                on_wait=list(pending_waits), on_update=[])
                        else:
                            si.on_wait = list(pending_waits) + list(si.on_wait)
                        pending_waits = []
                new.append(inst)
            assert not pending_waits
            bb.instructions[:] = new
    return n


def split_excess_waits(nc, max_waits=1):
    """This walrus build accepts only one sync-wait per instruction; split
    extra waits into preceding wait-only NoOps on the same engine."""
    n = 0
    for f in nc.m.functions:
        for bb in f.blocks:
            new = []
            for inst in bb.instructions:
                si = getattr(inst, "sync_info", None)
                waits = list(si.on_wait) if (si is not None and si.on_wait) else []
                if len(waits) > max_waits:
                    extra, keep = waits[:-max_waits], waits[-max_waits:]
                    for j, w in enumerate(extra):
                        new.append(mybir.InstNoOp(
                            name=f"{inst.name}_sw{j}",
                            engine=inst.engine,
                            bass_nofuse=True,
                            sync_info=mybir.SyncInfo(on_wait=[w], on_update=[]),
                        ))
                    si.on_wait = keep
                    n += 1
                new.append(inst)
            bb.instructions[:] = new
    return n


class Cfg:
    def __init__(self, S=2048, D=4096, H_LOC=4, CHUNK=512, n_cores=8):
        self.S = S              # sequence length
        self.D = D              # model dim (= contraction dim of projections)
        self.H_LOC = H_LOC      # heads per core
        self.CHUNK = CHUNK      # s-chunk size (outer loop granularity)
        self.n_cores = n_cores
        self.NK = D // P        # k-tiles in projections
        self.NCH = S // CHUNK   # number of s-chunks
        self.TPC = CHUNK // P   # s/t tiles per chunk (must be 4 for 512)
        self.DLOC = H_LOC * P   # local head dims
        self.SCALE = 1.0 / math.sqrt(P)  # 1/sqrt(hd)


FULL = Cfg()


def build_program(cfg: Cfg):
    """Builds the per-core Bass/Tile program (SPMD: same NEFF on all cores)."""
    S, NK, H_LOC, CHUNK, NCH, TPC = cfg.S, cfg.NK, cfg.H_LOC, cfg.CHUNK, cfg.NCH, cfg.TPC
    DLOC = cfg.DLOC

    nc = bass.Bass("TRN2", target_bir_lowering=False, debug=False,
                   num_devices=cfg.n_cores)

    # ---- DRAM I/O ----
    xt_d = nc.dram_tensor("xt", [NCH, 2, P, (NK // 2) * CHUNK], BF16,
                          kind="ExternalInput").ap()
    wq_d = nc.dram_tensor("wq", [H_LOC, P, NK * P], BF16, kind="ExternalInput").ap()
    wk_d = nc.dram_tensor("wk", [H_LOC, P, NK * P], BF16, kind="ExternalInput").ap()
    wv_d = nc.dram_tensor("wv", [P, NK, DLOC], BF16, kind="ExternalInput").ap()
    wo_d = nc.dram_tensor("wo", [cfg.D // CHUNK, P, H_LOC * CHUNK], BF16,
                          kind="ExternalInput").ap()
    cos_d = nc.dram_tensor("cosS", [P, S], BF16, kind="ExternalInput").ap()
    sin_d = nc.dram_tensor("sinm", [P, S], BF16, kind="ExternalInput").ap()
    triu_d = nc.dram_tensor("triu", [P, P], BF16, kind="ExternalInput").ap()
    ones_d = nc.dram_tensor("ones128", [P, P], BF16, kind="ExternalInput").ap()
    swp_d = nc.dram_tensor("swap128", [P, P], BF16, kind="ExternalInput").ap()
    ot_d = nc.dram_tensor("ot", [S, cfg.D], BF16, kind="ExternalOutput").ap()

    with tile.TileContext(nc) as tc:
        with tc.tile_pool(name="const", bufs=1) as const_pool, \
             tc.tile_pool(name="persist", bufs=1) as persist, \
             tc.tile_pool(name="xtp", bufs=2) as xtp, \
             tc.tile_pool(name="wqk", bufs=2) as wqkp, \
             tc.tile_pool(name="qtp", bufs=H_LOC + 3) as qtp, \
             tc.tile_pool(name="rawp", bufs=4) as rawp, \
             tc.tile_pool(name="pp", bufs=4 * TPC + 4) as pp, \
             tc.tile_pool(name="atp", bufs=H_LOC + 2) as atp, \
             tc.tile_pool(name="recp", bufs=2) as recp, \
             tc.tile_pool(name="osbp", bufs=4) as osbp, \
             tc.tile_pool(name="psA", bufs=2, space="PSUM") as psA, \
             tc.tile_pool(name="psS", bufs=2, space="PSUM") as psS, \
             tc.tile_pool(name="psAT", bufs=3, space="PSUM") as psAT, \
             tc.tile_pool(name="psR", bufs=1, space="PSUM") as psR:

            # constants (gpsimd DMA queue; small transfers, emitted after the
            # first W pieces so they don't delay the first matmul)
            triu = const_pool.tile([P, P], BF16, name="triu")
            ones128 = const_pool.tile([P, P], BF16, name="ones128")
            swap128 = const_pool.tile([P, P], BF16, name="swap128")
            cosS = const_pool.tile([P, S], BF16, name="cosS")
            sinm = const_pool.tile([P, S], BF16, name="sinm")

            def emit_cos_sin():
                nc.gpsimd.dma_start(swap128, swp_d)
                for j in range(4):
                    sl = slice(j * (S // 4), (j + 1) * (S // 4))
                    nc.gpsimd.dma_start(cosS[:, sl], cos_d[:, sl])
                    nc.gpsimd.dma_start(sinm[:, sl], sin_d[:, sl])
                nc.gpsimd.dma_start(triu, triu_d)
                nc.gpsimd.dma_start(ones128, ones_d)

            # persistent tensors: K^T per head, natural V, Wv, Wo
            KT = []
            for h in range(H_LOC):
                kt_h = persist.tile([P, S], BF16, name=f"kt{h}", tag=f"kt{h}")
                KT.append(kt_h)
            Vn = persist.tile([P, S // P, DLOC], BF16, name="vnat", tag="vnat")
            wv_pers = persist.tile([P, NK, DLOC], BF16, name="wv_pers",
                                   tag="wv_pers")
            NGR = cfg.D // CHUNK
            wo_pers = persist.tile([P, NGR, H_LOC, CHUNK], BF16,
                                   name="wo_pers", tag="wo_pers")

            def emit_wv():
                for q in range(8):
                    ksl = slice(q * (NK // 8), (q + 1) * (NK // 8))
                    nc.gpsimd.dma_start(wv_pers[:, ksl, :], wv_d[:, ksl, :])

            NKH = NK // 2
            xts_all = {}     # ch -> [half0, half1]
            qt_all = {}      # (ch, h) -> qt tile
            at_all = {}      # (ch, h) -> at tile

            def alloc_xt(ch):
                xts = [xtp.tile([P, NKH, CHUNK], BF16,
                                name=f"xt_{ch}_{half}", tag="xt")
                       for half in range(2)]
                xts_all[ch] = xts

            def emit_xt_quarter(ch, q8, fine=False):
                """One of 8 quarter-DMAs for chunk ch's x^T strip."""
                half, q = divmod(q8, 4)
                xh = xts_all[ch][half]
                src = xt_d[ch, half].rearrange("p (k c) -> p k c", c=CHUNK)
                kq = NKH // 4
                if fine:
                    for j in range(kq):
                        ksl = slice(q * kq + j, q * kq + j + 1)
                        nc.sync.dma_start(xh[:, ksl, :], src[:, ksl, :])
                else:
                    ksl = slice(q * kq, (q + 1) * kq)
                    nc.sync.dma_start(xh[:, ksl, :], src[:, ksl, :])

            def emit_xt(ch, fine=False):
                alloc_xt(ch)
                for q8 in range(8):
                    emit_xt_quarter(ch, q8, fine=(fine and q8 == 0))

            def emit_qk_unit(ch, which, h, after_w_hook=None):
                """One q/k projection unit: W load + NK matmuls + RoPE."""
                s0 = ch * CHUNK
                xts = xts_all[ch]
                w_dram = {"q": wq_d, "k": wk_d}[which]
                wt = wqkp.tile([P, NK, P], BF16,
                               name=f"w{which}_{ch}_{h}", tag="wqk")
                wsrc = w_dram[h].rearrange("p (k m) -> p k m", m=P)
                npieces = (8 if which == "q" else 4) if (ch == 0 and h == 0) else 2
                for q in range(npieces):
                    ksl = slice(q * (NK // npieces), (q + 1) * (NK // npieces))
                    nc.gpsimd.dma_start(wt[:, ksl, :], wsrc[:, ksl, :])
                if after_w_hook is not None:
                    after_w_hook()
                ps = psA.tile([P, CHUNK], F32,
                              name=f"ps_{which}_{ch}_{h}", tag="psA")
                for k in range(NK):
                    nc.tensor.matmul(ps, wt[:, k, :],
                                     xts[k // NKH][:, k % NKH, :],
                                     start=(k == 0), stop=(k == NK - 1))
                raw = rawp.tile([P, CHUNK], BF16,
                                name=f"raw_{which}_{ch}_{h}", tag="raw")
                nc.vector.tensor_copy(raw, ps)
                # RoPE: rot = raw*C2 + swap(raw)*S2m
                ps2 = psR.tile([P, CHUNK], F32,
                               name=f"psw_{which}_{ch}_{h}", tag="psR")
                nc.tensor.matmul(ps2, swap128, raw, start=True, stop=True)
                if which == "q":
                    dst = qtp.tile([P, CHUNK], BF16,
                                   name=f"qt_{ch}_{h}", tag="qt")
                    qt_all[(ch, h)] = dst
                else:
                    dst = KT[h][:, s0:s0 + CHUNK]
                tmp2 = rawp.tile([P, CHUNK], BF16,
                                 name=f"tmp2_{which}_{ch}_{h}", tag="tmp2")
                nc.vector.tensor_mul(dst, raw, cosS[:, s0:s0 + CHUNK])
                nc.vector.tensor_mul(tmp2, ps2, sinm[:, s0:s0 + CHUNK])
                nc.vector.tensor_add(dst, dst, tmp2)

            def emit_v_unit(ch, tl):
                """V projection for one t-tile, all heads, directly in natural
                [t, hd] layout: stationary = x^T block, moving = Wv columns."""
                xts = xts_all[ch]
                ps = psA.tile([P, DLOC], F32, name=f"psv_{ch}_{tl}", tag="psA")
                tsl = slice(tl * P, (tl + 1) * P)
                for k in range(NK):
                    nc.tensor.matmul(ps, xts[k // NKH][:, k % NKH, tsl],
                                     wv_pers[:, k, :],
                                     start=(k == 0), stop=(k == NK - 1))
                # V units run in outproj windows where ScalarE is idle
                nc.scalar.copy(Vn[:, ch * TPC + tl, :], ps)

            def emit_attn_head(ch, h):
                """Attention for (chunk, head): transposed scores -> exp ->
                PV, rowsum via ones-stationary pass, fused normalize."""
                n_t = (ch + 1) * TPC
                qt_h = qt_all[(ch, h)]
                psat = psAT.tile([P, CHUNK], F32, name=f"psat_{ch}_{h}",
                                 tag="psAT")
                pts = []
                pending = None
                for tb in range(n_t):
                    s_lo = max(0, tb - ch * TPC) * P
                    pss = psS.tile([P, CHUNK], F32,
                                   name=f"pss_{ch}_{h}_{tb}", tag="psS")
                    nc.tensor.matmul(pss[:, s_lo:], KT[h][:, tb * P:(tb + 1) * P],
                                     qt_h[:, s_lo:], start=True, stop=True)
                    pt = pp.tile([P, CHUNK], BF16,
                                 name=f"pt_{ch}_{h}_{tb}", tag="pt")
                    nc.scalar.activation(pt[:, s_lo:], pss[:, s_lo:],
                                         mybir.ActivationFunctionType.Exp,
                                         scale=cfg.SCALE)
                    if tb >= ch * TPC:
                        nc.vector.tensor_mul(pt[:, s_lo:s_lo + P],
                                             pt[:, s_lo:s_lo + P], triu)
                    if pending is not None:
                        ptb, plo, ppt = pending
                        nc.tensor.matmul(psat[:, plo:],
                                         Vn[:, ptb, h * P:(h + 1) * P],
                                         ppt[:, plo:],
                                         start=(ptb == 0), stop=False)
                    pending = (tb, s_lo, pt)
                    pts.append((tb, s_lo, pt))
                ptb, plo, ppt = pending
                nc.tensor.matmul(psat[:, plo:], Vn[:, ptb, h * P:(h + 1) * P],
                                 ppt[:, plo:], start=(ptb == 0), stop=True)
                # rowsum over t (partition axis) via all-ones stationary:
                # every output partition receives sum_t P^T[t, s] -- i.e. the
                # rowsum arrives pre-broadcast.  Consecutive matmuls share the
                # ones stationary (deduped to one ldweights).
                rs = psR.tile([P, CHUNK], F32, name=f"rs_{ch}_{h}", tag="psR")
                for tb, s_lo, pt in pts:
                    nc.tensor.matmul(rs[:, s_lo:], ones128, pt[:, s_lo:],
                                     start=(tb == 0), stop=(tb == n_t - 1))
                # 1/rowsum as exp(-ln(rowsum)) on ScalarE: both functions live
                # in the natural_log_exp table set (no table switching), and
                # the DVE reciprocal at [128,512] would cost 8 cyc/element.
                lnrs = recp.tile([P, CHUNK], F32,
                                 name=f"lnrs_{ch}_{h}", tag="lnrs")
                nc.scalar.activation(lnrs, rs,
                                     mybir.ActivationFunctionType.Ln)
                recipb = recp.tile([P, CHUNK], F32,
                                   name=f"rec_{ch}_{h}", tag="rec")
                nc.scalar.activation(recipb, lnrs,
                                     mybir.ActivationFunctionType.Exp,
                                     scale=-1.0)
                at_h = atp.tile([P, CHUNK], BF16, name=f"at_{ch}_{h}", tag="at")
                nc.vector.tensor_mul(at_h, psat, recipb)
                at_all[(ch, h)] = at_h

            def emit_outproj(ch, filler_units):
                """Output projection for s-chunk ch, interleaved with the
                given list of zero-arg emit callbacks (projection units /
                prefetches) so the PE never starves."""
                s0 = ch * CHUNK
                at_cur = [at_all[(ch, h)] for h in range(H_LOC)]
                nu = len(filler_units)
                NBLK = (NGR + 3) // 4
                n_iters = TPC * NBLK
                for it in range(n_iters):
                    stl, blk = divmod(it, NBLK)
                    ngs = list(range(blk * 4, min(blk * 4 + 4, NGR)))
                    psos = []
                    for j in range(len(ngs)):
                        pool, tg = (psS, "psS") if j < 2 else (psAT, "psAT")
                        pso = pool.tile([P, CHUNK], F32,
                                        name=f"pso_{ch}_{stl}_{blk}_{j}", tag=tg)
                        psos.append(pso)
                    for h in range(H_LOC):
                        lhs = at_cur[h][:, stl * P:(stl + 1) * P]
                        for j, ng in enumerate(ngs):
                            nc.tensor.matmul(psos[j], lhs, wo_pers[:, ng, h, :],
                                             start=(h == 0),
                                             stop=(h == H_LOC - 1))
                    for j, ng in enumerate(ngs):
                        osb = osbp.tile([P, CHUNK], BF16,
                                        name=f"osb_{ch}_{stl}_{blk}_{j}", tag="osb")
                        if j % 2:
                            nc.scalar.copy(osb, psos[j])
                        else:
                            nc.vector.tensor_copy(osb, psos[j])
                        srow = s0 + stl * P
                        if ch == NCH - 1 and it == n_iters - 1 and j % 2:
                            eng = nc.gpsimd
                        else:
                            eng = nc.sync
                        eng.dma_start(
                            ot_d[srow:srow + P, ng * CHUNK:(ng + 1) * CHUNK], osb)
                    for u in range(it * nu // n_iters,
                                   (it + 1) * nu // n_iters):
                        filler_units[u]()

            # ---------------- emission schedule ----------------
            emit_xt(0, fine=True)
            first = [True]

            def _cos_hook():
                if first[0]:
                    emit_cos_sin()
                    first[0] = False

            # chunk 0 projections, all up front.  GpSimd DMA queue order:
            # consts, Wq0+cos/sin, Wk0, Wv (4MB), remaining W, Wo -- each
            # lands just before its first consumer.  x^T(1) prefetches on the
            # SP queue behind x^T(0).
            emit_qk_unit(0, "q", 0, after_w_hook=_cos_hook)
            emit_qk_unit(0, "k", 0)
            emit_wv()
            if NCH > 1:
                emit_xt(1)
            for tl in range(TPC):
                emit_v_unit(0, tl)
            for h in range(1, H_LOC):
                emit_qk_unit(0, "q", h)
                emit_qk_unit(0, "k", h)

            def emit_wo():
                for ng in range(NGR):
                    nc.gpsimd.dma_start(
                        wo_pers[:, ng],
                        wo_d[ng].rearrange("p (h c) -> p h c", c=CHUNK))

            for ch in range(NCH):
                nxt = ch + 1
                # ---- attention window: heads of ch, interleaved with q/k
                # units of chunk ch+1 for heads 0..1 and x^T prefetch ----
                if 2 <= nxt < NCH:
                    # x^T(ch+1) prefetch; its buffers (chunk ch-1's) are
                    # long free -- proj(ch) finished last window
                    emit_xt(nxt)
                for h in range(H_LOC):
                    emit_attn_head(ch, h)
                    if nxt < NCH and h < 2:
                        emit_qk_unit(nxt, "q", h)
                        emit_qk_unit(nxt, "k", h)
                    if ch == 0 and h == min(1, H_LOC - 1):
                        # Wo load (4MB) behind the attn-window W units but
                        # well before outproj(0) needs it
                        emit_wo()
                # ---- output projection window: interleave v(ch+1) units and
                # the remaining q/k units of ch+1 ----
                fillers = []
                if nxt < NCH:
                    for tl in range(TPC):
                        fillers.append(lambda tl=tl: emit_v_unit(nxt, tl))
                    for h in range(2, H_LOC):
                        fillers.append(lambda h=h: emit_qk_unit(nxt, "q", h))
                        fillers.append(lambda h=h: emit_qk_unit(nxt, "k", h))
                emit_outproj(ch, fillers)

    dedup_ldweights(nc)
    split_excess_waits(nc)
    return nc


# ---------------- host-side data prep ----------------

def _tile_w(w_cols: np.ndarray, NK: int) -> np.ndarray:
    """[D, 128] per-head weight slice -> [128, NK*128] (k-part, k-outer*col)."""
    D = w_cols.shape[0]
    return np.ascontiguousarray(
        w_cols.reshape(NK, P, P).transpose(1, 0, 2).reshape(P, NK * P))


_ROPE_PERM = np.concatenate([np.arange(0, P, 2), np.arange(1, P, 2)])


def prepare_core_inputs(cfg: Cfg, core: int, x, wq, wk, wv, wo, cos, sin):
    """Builds the in_map (dict of numpy arrays) for one core."""
    bf = ml_dtypes.bfloat16
    S, D, H_LOC, CHUNK, NK, NCH = cfg.S, cfg.D, cfg.H_LOC, cfg.CHUNK, cfg.NK, cfg.NCH
    DLOC = cfg.DLOC
    c0 = core * DLOC

    out = {}
    # xt: [NCH, 2, 128, (NK//2)*CHUNK]
    xt = np.empty((NCH, 2, P, (NK // 2) * CHUNK), dtype=bf)
    xTb = x.T.astype(bf)  # [D, S]
    for ch in range(NCH):
        for half in range(2):
            blk = xTb[half * (D // 2):(half + 1) * (D // 2),
                      ch * CHUNK:(ch + 1) * CHUNK]          # [D/2, CHUNK]
            blk = blk.reshape(NK // 2, P, CHUNK).transpose(1, 0, 2)
            xt[ch, half] = blk.reshape(P, (NK // 2) * CHUNK)
    out["xt"] = xt

    for name, w in (("wq", wq), ("wk", wk)):
        wt = np.empty((H_LOC, P, NK * P), dtype=bf)
        for h in range(H_LOC):
            cols = w[:, c0 + h * P: c0 + (h + 1) * P][:, _ROPE_PERM]
            wt[h] = _tile_w(cols.astype(bf), NK)
        out[name] = wt

    # wv: [128, NK, DLOC]; wv_t[p, k, j] = wv[k*128+p, c0+j]
    wv_loc = wv[:, c0:c0 + DLOC].astype(bf)                  # [D, DLOC]
    out["wv"] = np.ascontiguousarray(
        wv_loc.reshape(NK, P, DLOC).transpose(1, 0, 2))

    # wo: [D//CHUNK, 128, H_LOC*CHUNK]; wo[ng, p, h*CHUNK+nl] = Wo[c0+h*128+p, ng*CHUNK+nl]
    wo_loc = wo[c0:c0 + DLOC, :].astype(bf)  # [DLOC, D]
    wo_t = np.empty((D // CHUNK, P, H_LOC * CHUNK), dtype=bf)
    for ng in range(D // CHUNK):
        blk = wo_loc[:, ng * CHUNK:(ng + 1) * CHUNK]     # [DLOC, CHUNK]
        blk = blk.reshape(H_LOC, P, CHUNK).transpose(1, 0, 2)
        wo_t[ng] = blk.reshape(P, H_LOC * CHUNK)
    out["wo"] = wo_t

    cosT = cos.T.astype(np.float32)    # [64, S]
    sinT = sin.T.astype(np.float32)
    out["cosS"] = np.concatenate([cosT, cosT], 0).astype(bf)
    out["sinm"] = np.concatenate([-sinT, sinT], 0).astype(bf)

    # triu (incl. diagonal) masks the diagonal 128-block of P^T [t, s]
    out["triu"] = np.triu(np.ones((P, P), np.float32)).astype(bf)
    out["ones128"] = np.ones((P, P), np.float32).astype(bf)
    sw = np.zeros((P, P), np.float32)
    sw[(np.arange(P) + 64) % P, np.arange(P)] = 1.0
    out["swap128"] = sw.astype(bf)
    return out


_PROGRAM_CACHE = {}


def get_program(cfg: Cfg):
    key = (cfg.S, cfg.D, cfg.H_LOC, cfg.CHUNK, cfg.n_cores)
    if key not in _PROGRAM_CACHE:
        _PROGRAM_CACHE[key] = build_program(cfg)
    return _PROGRAM_CACHE[key]


def run(cfg: Cfg, inputs: dict, trace: bool = False):
    """Run the sharded kernel; returns (list of per-core ot partials, results obj)."""
    install_ntff_hook_shim()
    enable_ldw_opt()
    x = np.asarray(inputs["x"], np.float32)
    wq = np.asarray(inputs["weight_q"], np.float32)
    wk = np.asarray(inputs["weight_k"], np.float32)
    wv = np.asarray(inputs["weight_v"], np.float32)
    wo = np.asarray(inputs["weight_o"], np.float32)
    cos = np.asarray(inputs["freqs_cos"], np.float32)
    sin = np.asarray(inputs["freqs_sin"], np.float32)

    nc = get_program(cfg)
    in_maps = [prepare_core_inputs(cfg, c, x, wq, wk, wv, wo, cos, sin)
               for c in range(cfg.n_cores)]
    res = bass_utils.run_bass_kernel_spmd(
        nc, in_maps, core_ids=list(range(cfg.n_cores)), trace=trace)
    return [r["ot"] for r in res.results], res


def kernel(**inputs) -> np.ndarray:
    ots, _ = run(FULL, inputs, trace=False)
    acc = np.zeros(ots[0].shape, dtype=np.float64)
    for ot in ots:
        acc += np.asarray(ot, dtype=np.float64)
    return np.ascontiguousarray(acc.astype(np.float32))


# revision 18
# speedup vs baseline: 1.0279x; 1.0034x over previous
"""Trainium2 Bass kernel for causal multi-head attention with RoPE
(nn_Attention: S=2048, D=4096, H=32, hd=128), tensor-parallel over heads
across 8 NeuronCores.

v2 strategy (per core, 4 heads):
  - Q^T/K^T projections head-major in [hd, S] layout (lhsT = W tile,
    rhs = x^T strip), bf16. RoPE via host-permuted [re;im] split:
    rot = raw*C2 + swap(raw)*S2m with a 128x128 swap matmul on the PE.
  - V projected DIRECTLY into natural [t, hd] layout: lhsT = x^T block
    [k,t-128] (stationary), rhs = Wv columns of all 4 heads [k, 512].
    No PE transposes for V; Wv is persistent in SBUF (loaded once).
  - Attention computes scores TRANSPOSED: scoresT[t, s-chunk] =
    (K^T tile)^T @ Q^T, so exp(scoresT) on ScalarE lands directly in the
    P^T layout that the PV matmul streams -- the per-block PE transposes
    of P from v1 are gone entirely.  Causal masking: t-tiles past the
    diagonal are skipped (ragged s_lo starts); the diagonal 128-block is
    masked multiplicatively (triu) on the DVE after exp.
  - Softmax denominators: rowsum over t is a partition-axis sum, done as
    one extra PE pass per head with an all-ones 128x128 stationary
    (ldweights deduped): out[p, s] = sum_t P^T[t, s] for every p, i.e.
    the rowsum is produced pre-broadcast across all partitions.  A
    single reciprocal_approx_fast (DVE) gives 1/rowsum, and the
    normalize is fused into the psum->sbuf copy of A^T (tensor_mul).
  - Output projection unchanged: O^T partial accumulated over the 4
    local heads, 4 concurrent psum groups sharing the stationary.
    Partials are written bf16; host sums the 8 partials in float64.

Scheduling: weight/const DMAs issue from the (otherwise idle) GpSimd
queue, x^T strips + outputs from SP.  ScalarE runs ONLY exp.  Emission
interleaves chunk ch's attention with chunk ch+1's q/k projections for
heads 0-1 and chunk ch's output projection with the remaining
projection units, so the PE stays fed through the Act-heavy late-chunk
attention windows.  x^T strips prefetch two windows ahead.
"""

import math
import sys
import types

import numpy as np
import ml_dtypes

import concourse.bass as bass
import concourse.tile as tile
import concourse.mybir as mybir
from concourse import bass_utils

BF16 = mybir.dt.bfloat16
F32 = mybir.dt.float32
P = 128


def enable_ldw_opt():
    """Flip walrus's --enable-ldw-opt to true (bass_utils hardcodes false).
    Patches run_command to rewrite the flag in the walrus argv."""
    import os
    if os.environ.get("BASS_LDW_OPT", "0") != "1":
        return
    if getattr(bass_utils, "_ldw_patch", False):
        return
    orig = bass_utils.run_command

    def patched(argv, **kwargs):
        argv = ["--enable-ldw-opt=true" if a == "--enable-ldw-opt=false" else a
                for a in argv]
        return orig(argv, **kwargs)

    bass_utils.run_command = patched
    bass_utils._ldw_patch = True


def install_ntff_hook_shim():
    """Make trace=True work under axon (antenv.axon_hooks is absent here)."""
    try:
        import antenv.axon_hooks  # noqa
        return
    except ImportError:
        pass
    try:
        import antenv
        from trn_agent_boot.trn_boot import _ntff_profile_via_ctypes
        hook = _ntff_profile_via_ctypes('/opt/axon/libaxon_pjrt.so')
        mod = types.ModuleType('antenv.axon_hooks')
        mod.get_axon_ntff_profile_hook = lambda: hook
        mod.set_axon_ntff_profile_hook = lambda h: None
        sys.modules['antenv.axon_hooks'] = mod
        antenv.axon_hooks = mod
    except Exception:
        pass


def dedup_ldweights(nc):
    """Remove an InstLdweights when the immediately preceding PE weight load
    has an identical stationary operand (consecutive matmuls sharing lhsT).
    Any waits on the removed load are transferred to the next instruction."""
    import concourse.mybir as _mb
    n = 0
    for f in nc.m.functions:
        for bb in f.blocks:
            new = []
            last_key = None
            pending_waits = []
            for inst in bb.instructions:
                ty = type(inst).__name__
                eng = getattr(inst, "engine", None)
                if eng == _mb.EngineType.PE:
                    if ty == "InstLdweights":
                        o = inst.ins[0]
                        key = (str(getattr(o, "memref", "")), o.offset,
                               str(o.ap), str(getattr(o, "dtype", "")),
                               getattr(inst, "is_transpose", None),
                               getattr(inst, "tile_position", None))
                        if key == last_key:
                            si = getattr(inst, "sync_info", None)
                            if si is not None and si.on_wait:
                                pending_waits.extend(si.on_wait)
                            n += 1
                            continue   # drop this load
                        last_key = key
                    elif ty in ("InstMatmult", "InstEventSemaphore", "InstNoOp"):
                        pass           # none of these clobber loaded weights
                    else:
                        last_key = None
                    if pending_waits:
                        si = getattr(inst, "sync_info", None)
                        if si is None:
                            inst.sync_info = _mb.SyncInfo(
                                on_wait=list(pending_waits), on_update=[])
                        else:
                            si.on_wait = list(pending_waits) + list(si.on_wait)
                        pending_waits = []
                new.append(inst)
            assert not pending_waits
            bb.instructions[:] = new
    return n


def split_excess_waits(nc, max_waits=1):
    """This walrus build accepts only one sync-wait per instruction; split
    extra waits into preceding wait-only NoOps on the same engine."""
    n = 0
    for f in nc.m.functions:
        for bb in f.blocks:
            new = []
            for inst in bb.instructions:
                si = getattr(inst, "sync_info", None)
                waits = list(si.on_wait) if (si is not None and si.on_wait) else []
                if len(waits) > max_waits:
                    extra, keep = waits[:-max_waits], waits[-max_waits:]
                    for j, w in enumerate(extra):
                        new.append(mybir.InstNoOp(
                            name=f"{inst.name}_sw{j}",
                            engine=inst.engine,
                            bass_nofuse=True,
                            sync_info=mybir.SyncInfo(on_wait=[w], on_update=[]),
                        ))
                    si.on_wait = keep
                    n += 1
                new.append(inst)
            bb.instructions[:] = new
    return n


class Cfg:
    def __init__(self, S=2048, D=4096, H_LOC=4, CHUNK=512, n_cores=8):
        self.S = S              # sequence length
        self.D = D              # model dim (= contraction dim of projections)
        self.H_LOC = H_LOC      # heads per core
        self.CHUNK = CHUNK      # s-chunk size (outer loop granularity)
        self.n_cores = n_cores
        self.NK = D // P        # k-tiles in projections
        self.NCH = S // CHUNK   # number of s-chunks
        self.TPC = CHUNK // P   # s/t tiles per chunk (must be 4 for 512)
        self.DLOC = H_LOC * P   # local head dims
        self.SCALE = 1.0 / math.sqrt(P)  # 1/sqrt(hd)


FULL = Cfg()


def build_program(cfg: Cfg):
    """Builds the per-core Bass/Tile program (SPMD: same NEFF on all cores)."""
    S, NK, H_LOC, CHUNK, NCH, TPC = cfg.S, cfg.NK, cfg.H_LOC, cfg.CHUNK, cfg.NCH, cfg.TPC
    DLOC = cfg.DLOC

    nc = bass.Bass("TRN2", target_bir_lowering=False, debug=False,
                   num_devices=cfg.n_cores)

    # ---- DRAM I/O ----
    xt_d = nc.dram_tensor("xt", [NCH, 2, P, (NK // 2) * CHUNK], BF16,
                          kind="ExternalInput").ap()
    wq_d = nc.dram_tensor("wq", [H_LOC, P, NK * P], BF16, kind="ExternalInput").ap()
    wk_d = nc.dram_tensor("wk", [H_LOC, P, NK * P], BF16, kind="ExternalInput").ap()
    wv_d = nc.dram_tensor("wv", [P, NK, DLOC], BF16, kind="ExternalInput").ap()
    wo_d = nc.dram_tensor("wo", [cfg.D // CHUNK, P, H_LOC * CHUNK], BF16,
                          kind="ExternalInput").ap()
    cos_d = nc.dram_tensor("cosS", [P, S], BF16, kind="ExternalInput").ap()
    sin_d = nc.dram_tensor("sinm", [P, S], BF16, kind="ExternalInput").ap()
    triu_d = nc.dram_tensor("triu", [P, P], BF16, kind="ExternalInput").ap()
    ones_d = nc.dram_tensor("ones128", [P, P], BF16, kind="ExternalInput").ap()
    swp_d = nc.dram_tensor("swap128", [P, P], BF16, kind="ExternalInput").ap()
    ot_d = nc.dram_tensor("ot", [S, cfg.D], BF16, kind="ExternalOutput").ap()

    with tile.TileContext(nc) as tc:
        with tc.tile_pool(name="const", bufs=1) as const_pool, \
             tc.tile_pool(name="persist", bufs=1) as persist, \
             tc.tile_pool(name="xtp", bufs=2) as xtp, \
             tc.tile_pool(name="wqk", bufs=2) as wqkp, \
             tc.tile_pool(name="qtp", bufs=H_LOC + 3) as qtp, \
             tc.tile_pool(name="rawp", bufs=4) as rawp, \
             tc.tile_pool(name="pp", bufs=4 * TPC + 4) as pp, \
             tc.tile_pool(name="atp", bufs=H_LOC + 2) as atp, \
             tc.tile_pool(name="recp", bufs=2) as recp, \
             tc.tile_pool(name="osbp", bufs=4) as osbp, \
             tc.tile_pool(name="psA", bufs=2, space="PSUM") as psA, \
             tc.tile_pool(name="psS", bufs=2, space="PSUM") as psS, \
             tc.tile_pool(name="psAT", bufs=3, space="PSUM") as psAT, \
             tc.tile_pool(name="psR", bufs=1, space="PSUM") as psR:

            # constants (gpsimd DMA queue; small transfers, emitted after the
            # first W pieces so they don't delay the first matmul)
            triu = const_pool.tile([P, P], BF16, name="triu")
            ones128 = const_pool.tile([P, P], BF16, name="ones128")
            swap128 = const_pool.tile([P, P], BF16, name="swap128")
            cosS = const_pool.tile([P, S], BF16, name="cosS")
            sinm = const_pool.tile([P, S], BF16, name="sinm")

            def emit_cos_sin():
                nc.gpsimd.dma_start(swap128, swp_d)
                for j in range(4):
                    sl = slice(j * (S // 4), (j + 1) * (S // 4))
                    nc.gpsimd.dma_start(cosS[:, sl], cos_d[:, sl])
                    nc.gpsimd.dma_start(sinm[:, sl], sin_d[:, sl])
                nc.gpsimd.dma_start(triu, triu_d)
                nc.gpsimd.dma_start(ones128, ones_d)

            # persistent tensors: K^T per head, natural V, Wv, Wo
            KT = []
            for h in range(H_LOC):
                kt_h = persist.tile([P, S], BF16, name=f"kt{h}", tag=f"kt{h}")
                KT.append(kt_h)
            Vn = persist.tile([P, S // P, DLOC], BF16, name="vnat", tag="vnat")
            wv_pers = persist.tile([P, NK, DLOC], BF16, name="wv_pers",
                                   tag="wv_pers")
            NGR = cfg.D // CHUNK
            wo_pers = persist.tile([P, NGR, H_LOC, CHUNK], BF16,
                                   name="wo_pers", tag="wo_pers")

            def emit_wv():
                for q in range(8):
                    ksl = slice(q * (NK // 8), (q + 1) * (NK // 8))
                    nc.gpsimd.dma_start(wv_pers[:, ksl, :], wv_d[:, ksl, :])

            NKH = NK // 2
            xts_all = {}     # ch -> [half0, half1]
            qt_all = {}      # (ch, h) -> qt tile
            at_all = {}      # (ch, h) -> at tile

            def alloc_xt(ch):
                xts = [xtp.tile([P, NKH, CHUNK], BF16,
                                name=f"xt_{ch}_{half}", tag="xt")
                       for half in range(2)]
                xts_all[ch] = xts

            def emit_xt_quarter(ch, q8, fine=False):
                """One of 8 quarter-DMAs for chunk ch's x^T strip."""
                half, q = divmod(q8, 4)
                xh = xts_all[ch][half]
                src = xt_d[ch, half].rearrange("p (k c) -> p k c", c=CHUNK)
                kq = NKH // 4
                if fine:
                    for j in range(kq):
                        ksl = slice(q * kq + j, q * kq + j + 1)
                        nc.sync.dma_start(xh[:, ksl, :], src[:, ksl, :])
                else:
                    ksl = slice(q * kq, (q + 1) * kq)
                    nc.sync.dma_start(xh[:, ksl, :], src[:, ksl, :])

            def emit_xt(ch, fine=False):
                alloc_xt(ch)
                for q8 in range(8):
                    emit_xt_quarter(ch, q8, fine=(fine and q8 == 0))

            def emit_qk_unit(ch, which, h, after_w_hook=None):
                """One q/k projection unit: W load + NK matmuls + RoPE."""
                s0 = ch * CHUNK
                xts = xts_all[ch]
                w_dram = {"q": wq_d, "k": wk_d}[which]
                wt = wqkp.tile([P, NK, P], BF16,
                               name=f"w{which}_{ch}_{h}", tag="wqk")
                wsrc = w_dram[h].rearrange("p (k m) -> p k m", m=P)
                npieces = (8 if which == "q" else 4) if (ch == 0 and h == 0) else 2
                for q in range(npieces):
                    ksl = slice(q * (NK // npieces), (q + 1) * (NK // npieces))
                    nc.gpsimd.dma_start(wt[:, ksl, :], wsrc[:, ksl, :])
                if after_w_hook is not None:
                    after_w_hook()
                ps = psA.tile([P, CHUNK], F32,
                              name=f"ps_{which}_{ch}_{h}", tag="psA")
                for k in range(NK):
                    nc.tensor.matmul(ps, wt[:, k, :],
                                     xts[k // NKH][:, k % NKH, :],
                                     start=(k == 0), stop=(k == NK - 1))
                raw = rawp.tile([P, CHUNK], BF16,
                                name=f"raw_{which}_{ch}_{h}", tag="raw")
                nc.vector.tensor_copy(raw, ps)
                # RoPE: rot = raw*C2 + swap(raw)*S2m
                ps2 = psR.tile([P, CHUNK], F32,
                               name=f"psw_{which}_{ch}_{h}", tag="psR")
                nc.tensor.matmul(ps2, swap128, raw, start=True, stop=True)
                if which == "q":
                    dst = qtp.tile([P, CHUNK], BF16,
                                   name=f"qt_{ch}_{h}", tag="qt")
                    qt_all[(ch, h)] = dst
                else:
                    dst = KT[h][:, s0:s0 + CHUNK]
                tmp2 = rawp.tile([P, CHUNK], BF16,
                                 name=f"tmp2_{which}_{ch}_{h}", tag="tmp2")
                nc.vector.tensor_mul(dst, raw, cosS[:, s0:s0 + CHUNK])
                nc.vector.tensor_mul(tmp2, ps2, sinm[:, s0:s0 + CHUNK])
                nc.vector.tensor_add(dst, dst, tmp2)

            def emit_v_unit(ch, tl):
                """V projection for one t-tile, all heads, directly in natural
                [t, hd] layout: stationary = x^T block, moving = Wv columns."""
                xts = xts_all[ch]
                ps = psA.tile([P, DLOC], F32, name=f"psv_{ch}_{tl}", tag="psA")
                tsl = slice(tl * P, (tl + 1) * P)
                for k in range(NK):
                    nc.tensor.matmul(ps, xts[k // NKH][:, k % NKH, tsl],
                                     wv_pers[:, k, :],
                                     start=(k == 0), stop=(k == NK - 1))
                # V units run in outproj windows where ScalarE is idle
                nc.scalar.copy(Vn[:, ch * TPC + tl, :], ps)

            def emit_attn_head(ch, h):
                """Attention for (chunk, head): transposed scores -> exp ->
                PV, rowsum via ones-stationary pass, fused normalize."""
                n_t = (ch + 1) * TPC
                qt_h = qt_all[(ch, h)]
                psat = psAT.tile([P, CHUNK], F32, name=f"psat_{ch}_{h}",
                                 tag="psAT")
                pts = []
                pending = None
                for tb in range(n_t):
                    s_lo = max(0, tb - ch * TPC) * P
                    pss = psS.tile([P, CHUNK], F32,
                                   name=f"pss_{ch}_{h}_{tb}", tag="psS")
                    nc.tensor.matmul(pss[:, s_lo:], KT[h][:, tb * P:(tb + 1) * P],
                                     qt_h[:, s_lo:], start=True, stop=True)
                    pt = pp.tile([P, CHUNK], BF16,
                                 name=f"pt_{ch}_{h}_{tb}", tag="pt")
                    nc.scalar.activation(pt[:, s_lo:], pss[:, s_lo:],
                                         mybir.ActivationFunctionType.Exp,
                                         scale=cfg.SCALE)
                    if tb >= ch * TPC:
                        nc.vector.tensor_mul(pt[:, s_lo:s_lo + P],
                                             pt[:, s_lo:s_lo + P], triu)
                    if pending is not None:
                        ptb, plo, ppt = pending
                        nc.tensor.matmul(psat[:, plo:],
                                         Vn[:, ptb, h * P:(h + 1) * P],
                                         ppt[:, plo:],
                                         start=(ptb == 0), stop=False)
                    pending = (tb, s_lo, pt)
                    pts.append((tb, s_lo, pt))
                ptb, plo, ppt = pending
                nc.tensor.matmul(psat[:, plo:], Vn[:, ptb, h * P:(h + 1) * P],
                                 ppt[:, plo:], start=(ptb == 0), stop=True)
                # rowsum over t (partition axis) via all-ones stationary:
                # every output partition receives sum_t P^T[t, s] -- i.e. the
                # rowsum arrives pre-broadcast.  Consecutive matmuls share the
                # ones stationary (deduped to one ldweights).
                rs = psR.tile([P, CHUNK], F32, name=f"rs_{ch}_{h}", tag="psR")
                for tb, s_lo, pt in pts:
                    nc.tensor.matmul(rs[:, s_lo:], ones128, pt[:, s_lo:],
                                     start=(tb == 0), stop=(tb == n_t - 1))
                # 1/rowsum as exp(-ln(rowsum)) on ScalarE: both functions live
                # in the natural_log_exp table set (no table switching), and
                # the DVE reciprocal at [128,512] would cost 8 cyc/element.
                lnrs = recp.tile([P, CHUNK], F32,
                                 name=f"lnrs_{ch}_{h}", tag="lnrs")
                nc.scalar.activation(lnrs, rs,
                                     mybir.ActivationFunctionType.Ln)
                recipb = recp.tile([P, CHUNK], F32,
                                   name=f"rec_{ch}_{h}", tag="rec")
                nc.scalar.activation(recipb, lnrs,
                                     mybir.ActivationFunctionType.Exp,
                                     scale=-1.0)
                at_h = atp.tile([P, CHUNK], BF16, name=f"at_{ch}_{h}", tag="at")
                nc.vector.tensor_mul(at_h, psat, recipb)
                at_all[(ch, h)] = at_h

            def emit_outproj(ch, filler_units):
                """Output projection for s-chunk ch, interleaved with the
                given list of zero-arg emit callbacks (projection units /
                prefetches) so the PE never starves."""
                s0 = ch * CHUNK
                at_cur = [at_all[(ch, h)] for h in range(H_LOC)]
                nu = len(filler_units)
                NBLK = (NGR + 3) // 4
                n_iters = TPC * NBLK
                for it in range(n_iters):
                    stl, blk = divmod(it, NBLK)
                    ngs = list(range(blk * 4, min(blk * 4 + 4, NGR)))
                    psos = []
                    for j in range(len(ngs)):
                        pool, tg = (psS, "psS") if j < 2 else (psAT, "psAT")
                        pso = pool.tile([P, CHUNK], F32,
                                        name=f"pso_{ch}_{stl}_{blk}_{j}", tag=tg)
                        psos.append(pso)
                    for h in range(H_LOC):
                        lhs = at_cur[h][:, stl * P:(stl + 1) * P]
                        for j, ng in enumerate(ngs):
                            nc.tensor.matmul(psos[j], lhs, wo_pers[:, ng, h, :],
                                             start=(h == 0),
                                             stop=(h == H_LOC - 1))
                    for j, ng in enumerate(ngs):
                        osb = osbp.tile([P, CHUNK], BF16,
                                        name=f"osb_{ch}_{stl}_{blk}_{j}", tag="osb")
                        if j % 2:
                            nc.scalar.copy(osb, psos[j])
                        else:
                            nc.vector.tensor_copy(osb, psos[j])
                        srow = s0 + stl * P
                        if ch == NCH - 1 and it == n_iters - 1 and j % 2:
                            eng = nc.gpsimd
                        else:
                            eng = nc.sync
                        eng.dma_start(
                            ot_d[srow:srow + P, ng * CHUNK:(ng + 1) * CHUNK], osb)
                    for u in range(it * nu // n_iters,
                                   (it + 1) * nu // n_iters):
                        filler_units[u]()

            # ---------------- emission schedule ----------------
            emit_xt(0, fine=True)
            first = [True]

            def _cos_hook():
                if first[0]:
                    emit_cos_sin()
                    first[0] = False

            # chunk 0 projections, all up front.  GpSimd DMA queue order:
            # consts, Wq0+cos/sin, Wk0, Wv (4MB), remaining W, Wo -- each
            # lands just before its first consumer.  x^T(1) prefetches on the
            # SP queue behind x^T(0).
            emit_qk_unit(0, "q", 0, after_w_hook=_cos_hook)
            emit_qk_unit(0, "k", 0)
            emit_wv()
            if NCH > 1:
                emit_xt(1)
            for tl in range(TPC):
                emit_v_unit(0, tl)
            for h in range(1, H_LOC):
                emit_qk_unit(0, "q", h)
                emit_qk_unit(0, "k", h)

            for ng in range(NGR):
                nc.gpsimd.dma_start(
                    wo_pers[:, ng],
                    wo_d[ng].rearrange("p (h c) -> p h c", c=CHUNK))

            for ch in range(NCH):
                nxt = ch + 1
                # ---- attention window: heads of ch, interleaved with q/k
                # units of chunk ch+1 for heads 0..1 and x^T prefetch ----
                if 2 <= nxt < NCH:
                    # x^T(ch+1) prefetch; its buffers (chunk ch-1's) are
                    # long free -- proj(ch) finished last window
                    emit_xt(nxt)
                for h in range(H_LOC):
                    emit_attn_head(ch, h)
                    if nxt < NCH and h < 2:
                        emit_qk_unit(nxt, "q", h)
                        emit_qk_unit(nxt, "k", h)
                # ---- output projection window: interleave v(ch+1) units and
                # the remaining q/k units of ch+1 ----
                fillers = []
                if nxt < NCH:
                    for tl in range(TPC):
                        fillers.append(lambda tl=tl: emit_v_unit(nxt, tl))
                    for h in range(2, H_LOC):
                        fillers.append(lambda h=h: emit_qk_unit(nxt, "q", h))
                        fillers.append(lambda h=h: emit_qk_unit(nxt, "k", h))
                emit_outproj(ch, fillers)

    dedup_ldweights(nc)
    split_excess_waits(nc)
    return nc


# ---------------- host-side data prep ----------------

def _tile_w(w_cols: np.ndarray, NK: int) -> np.ndarray:
    """[D, 128] per-head weight slice -> [128, NK*128] (k-part, k-outer*col)."""
    D = w_cols.shape[0]
    return np.ascontiguousarray(
        w_cols.reshape(NK, P, P).transpose(1, 0, 2).reshape(P, NK * P))


_ROPE_PERM = np.concatenate([np.arange(0, P, 2), np.arange(1, P, 2)])


def prepare_core_inputs(cfg: Cfg, core: int, x, wq, wk, wv, wo, cos, sin):
    """Builds the in_map (dict of numpy arrays) for one core."""
    bf = ml_dtypes.bfloat16
    S, D, H_LOC, CHUNK, NK, NCH = cfg.S, cfg.D, cfg.H_LOC, cfg.CHUNK, cfg.NK, cfg.NCH
    DLOC = cfg.DLOC
    c0 = core * DLOC

    out = {}
    # xt: [NCH, 2, 128, (NK//2)*CHUNK]
    xt = np.empty((NCH, 2, P, (NK // 2) * CHUNK), dtype=bf)
    xTb = x.T.astype(bf)  # [D, S]
    for ch in range(NCH):
        for half in range(2):
            blk = xTb[half * (D // 2):(half + 1) * (D // 2),
                      ch * CHUNK:(ch + 1) * CHUNK]          # [D/2, CHUNK]
            blk = blk.reshape(NK // 2, P, CHUNK).transpose(1, 0, 2)
            xt[ch, half] = blk.reshape(P, (NK // 2) * CHUNK)
    out["xt"] = xt

    for name, w in (("wq", wq), ("wk", wk)):
        wt = np.empty((H_LOC, P, NK * P), dtype=bf)
        for h in range(H_LOC):
            cols = w[:, c0 + h * P: c0 + (h + 1) * P][:, _ROPE_PERM]
            wt[h] = _tile_w(cols.astype(bf), NK)
        out[name] = wt

    # wv: [128, NK, DLOC]; wv_t[p, k, j] = wv[k*128+p, c0+j]
    wv_loc = wv[:, c0:c0 + DLOC].astype(bf)                  # [D, DLOC]
    out["wv"] = np.ascontiguousarray(
        wv_loc.reshape(NK, P, DLOC).transpose(1, 0, 2))

    # wo: [D//CHUNK, 128, H_LOC*CHUNK]; wo[ng, p, h*CHUNK+nl] = Wo[c0+h*128+p, ng*CHUNK+nl]
    wo_loc = wo[c0:c0 + DLOC, :].astype(bf)  # [DLOC, D]
    wo_t = np.empty((D // CHUNK, P, H_LOC * CHUNK), dtype=bf)
    for ng in range(D // CHUNK):
        blk = wo_loc[:, ng * CHUNK:(ng + 1) * CHUNK]     # [DLOC, CHUNK]
        blk = blk.reshape(H_LOC, P, CHUNK).transpose(1, 0, 2)
        wo_t[ng] = blk.reshape(P, H_LOC * CHUNK)
    out["wo"] = wo_t

    cosT = cos.T.astype(np.float32)    # [64, S]
    sinT = sin.T.astype(np.float32)
    out["cosS"] = np.concatenate([cosT, cosT], 0).astype(bf)
    out["sinm"] = np.concatenate([-sinT, sinT], 0).astype(bf)

    # triu (incl. diagonal) masks the diagonal 128-block of P^T [t, s]
    out["triu"] = np.triu(np.ones((P, P), np.float32)).astype(bf)
    out["ones128"] = np.ones((P, P), np.float32).astype(bf)
    sw = np.zeros((P, P), np.float32)
    sw[(np.arange(P) + 64) % P, np.arange(P)] = 1.0
    out["swap128"] = sw.astype(bf)
    return out


_PROGRAM_CACHE = {}


def get_program(cfg: Cfg):
    key = (cfg.S, cfg.D, cfg.H_LOC, cfg.CHUNK, cfg.n_cores)
    if key not in _PROGRAM_CACHE:
        _PROGRAM_CACHE[key] = build_program(cfg)
    return _PROGRAM_CACHE[key]


def run(cfg: Cfg, inputs: dict, trace: bool = False):
    """Run the sharded kernel; returns (list of per-core ot partials, results obj)."""
    install_ntff_hook_shim()
    enable_ldw_opt()
    x = np.asarray(inputs["x"], np.float32)
    wq = np.asarray(inputs["weight_q"], np.float32)
    wk = np.asarray(inputs["weight_k"], np.float32)
    wv = np.asarray(inputs["weight_v"], np.float32)
    wo = np.asarray(inputs["weight_o"], np.float32)
    cos = np.asarray(inputs["freqs_cos"], np.float32)
    sin = np.asarray(inputs["freqs_sin"], np.float32)

    nc = get_program(cfg)
    in_maps = [prepare_core_inputs(cfg, c, x, wq, wk, wv, wo, cos, sin)
               for c in range(cfg.n_cores)]
    res = bass_utils.run_bass_kernel_spmd(
        nc, in_maps, core_ids=list(range(cfg.n_cores)), trace=trace)
    return [r["ot"] for r in res.results], res


def kernel(**inputs) -> np.ndarray:
    ots, _ = run(FULL, inputs, trace=False)
    acc = np.zeros(ots[0].shape, dtype=np.float64)
    for ot in ots:
        acc += np.asarray(ot, dtype=np.float64)
    return np.ascontiguousarray(acc.astype(np.float32))


# revision 19
# speedup vs baseline: 1.0424x; 1.0141x over previous
"""Trainium2 Bass kernel for causal multi-head attention with RoPE
(nn_Attention: S=2048, D=4096, H=32, hd=128), tensor-parallel over heads
across 8 NeuronCores.

v2 strategy (per core, 4 heads):
  - Q^T/K^T projections head-major in [hd, S] layout (lhsT = W tile,
    rhs = x^T strip), bf16. RoPE via host-permuted [re;im] split:
    rot = raw*C2 + swap(raw)*S2m with a 128x128 swap matmul on the PE.
  - V projected DIRECTLY into natural [t, hd] layout: lhsT = x^T block
    [k,t-128] (stationary), rhs = Wv columns of all 4 heads [k, 512].
    No PE transposes for V; Wv is persistent in SBUF (loaded once).
  - Attention computes scores TRANSPOSED: scoresT[t, s-chunk] =
    (K^T tile)^T @ Q^T, so exp(scoresT) on ScalarE lands directly in the
    P^T layout that the PV matmul streams -- the per-block PE transposes
    of P from v1 are gone entirely.  Causal masking: t-tiles past the
    diagonal are skipped (ragged s_lo starts); the diagonal 128-block is
    masked multiplicatively (triu) on the DVE after exp.
  - Softmax denominators: rowsum over t is a partition-axis sum, done as
    one extra PE pass per head with an all-ones 128x128 stationary
    (ldweights deduped): out[p, s] = sum_t P^T[t, s] for every p, i.e.
    the rowsum is produced pre-broadcast across all partitions.  A
    single reciprocal_approx_fast (DVE) gives 1/rowsum, and the
    normalize is fused into the psum->sbuf copy of A^T (tensor_mul).
  - Output projection unchanged: O^T partial accumulated over the 4
    local heads, 4 concurrent psum groups sharing the stationary.
    Partials are written bf16; host sums the 8 partials in float64.

Scheduling: weight/const DMAs issue from the (otherwise idle) GpSimd
queue, x^T strips + outputs from SP.  ScalarE runs ONLY exp.  Emission
interleaves chunk ch's attention with chunk ch+1's q/k projections for
heads 0-1 and chunk ch's output projection with the remaining
projection units, so the PE stays fed through the Act-heavy late-chunk
attention windows.  x^T strips prefetch two windows ahead.
"""

import math
import sys
import types

import numpy as np
import ml_dtypes

import concourse.bass as bass
import concourse.tile as tile
import concourse.mybir as mybir
from concourse import bass_utils

BF16 = mybir.dt.bfloat16
F32 = mybir.dt.float32
P = 128


def enable_ldw_opt():
    """Flip walrus's --enable-ldw-opt to true (bass_utils hardcodes false).
    Patches run_command to rewrite the flag in the walrus argv."""
    import os
    if os.environ.get("BASS_LDW_OPT", "0") != "1":
        return
    if getattr(bass_utils, "_ldw_patch", False):
        return
    orig = bass_utils.run_command

    def patched(argv, **kwargs):
        argv = ["--enable-ldw-opt=true" if a == "--enable-ldw-opt=false" else a
                for a in argv]
        return orig(argv, **kwargs)

    bass_utils.run_command = patched
    bass_utils._ldw_patch = True


def install_ntff_hook_shim():
    """Make trace=True work under axon (antenv.axon_hooks is absent here)."""
    try:
        import antenv.axon_hooks  # noqa
        return
    except ImportError:
        pass
    try:
        import antenv
        from trn_agent_boot.trn_boot import _ntff_profile_via_ctypes
        hook = _ntff_profile_via_ctypes('/opt/axon/libaxon_pjrt.so')
        mod = types.ModuleType('antenv.axon_hooks')
        mod.get_axon_ntff_profile_hook = lambda: hook
        mod.set_axon_ntff_profile_hook = lambda h: None
        sys.modules['antenv.axon_hooks'] = mod
        antenv.axon_hooks = mod
    except Exception:
        pass


def dedup_ldweights(nc):
    """Remove an InstLdweights when the immediately preceding PE weight load
    has an identical stationary operand (consecutive matmuls sharing lhsT).
    Any waits on the removed load are transferred to the next instruction."""
    import concourse.mybir as _mb
    n = 0
    for f in nc.m.functions:
        for bb in f.blocks:
            new = []
            last_key = None
            pending_waits = []
            for inst in bb.instructions:
                ty = type(inst).__name__
                eng = getattr(inst, "engine", None)
                if eng == _mb.EngineType.PE:
                    if ty == "InstLdweights":
                        o = inst.ins[0]
                        key = (str(getattr(o, "memref", "")), o.offset,
                               str(o.ap), str(getattr(o, "dtype", "")),
                               getattr(inst, "is_transpose", None),
                               getattr(inst, "tile_position", None))
                        if key == last_key:
                            si = getattr(inst, "sync_info", None)
                            if si is not None and si.on_wait:
                                pending_waits.extend(si.on_wait)
                            n += 1
                            continue   # drop this load
                        last_key = key
                    elif ty in ("InstMatmult", "InstEventSemaphore", "InstNoOp"):
                        pass           # none of these clobber loaded weights
                    else:
                        last_key = None
                    if pending_waits:
                        si = getattr(inst, "sync_info", None)
                        if si is None:
                            inst.sync_info = _mb.SyncInfo(
                                on_wait=list(pending_waits), on_update=[])
                        else:
                            si.on_wait = list(pending_waits) + list(si.on_wait)
                        pending_waits = []
                new.append(inst)
            assert not pending_waits
            bb.instructions[:] = new
    return n


def split_excess_waits(nc, max_waits=1):
    """This walrus build accepts only one sync-wait per instruction; split
    extra waits into preceding wait-only NoOps on the same engine."""
    n = 0
    for f in nc.m.functions:
        for bb in f.blocks:
            new = []
            for inst in bb.instructions:
                si = getattr(inst, "sync_info", None)
                waits = list(si.on_wait) if (si is not None and si.on_wait) else []
                if len(waits) > max_waits:
                    extra, keep = waits[:-max_waits], waits[-max_waits:]
                    for j, w in enumerate(extra):
                        new.append(mybir.InstNoOp(
                            name=f"{inst.name}_sw{j}",
                            engine=inst.engine,
                            bass_nofuse=True,
                            sync_info=mybir.SyncInfo(on_wait=[w], on_update=[]),
                        ))
                    si.on_wait = keep
                    n += 1
                new.append(inst)
            bb.instructions[:] = new
    return n


class Cfg:
    def __init__(self, S=2048, D=4096, H_LOC=4, CHUNK=512, n_cores=8):
        self.S = S              # sequence length
        self.D = D              # model dim (= contraction dim of projections)
        self.H_LOC = H_LOC      # heads per core
        self.CHUNK = CHUNK      # s-chunk size (outer loop granularity)
        self.n_cores = n_cores
        self.NK = D // P        # k-tiles in projections
        self.NCH = S // CHUNK   # number of s-chunks
        self.TPC = CHUNK // P   # s/t tiles per chunk (must be 4 for 512)
        self.DLOC = H_LOC * P   # local head dims
        self.SCALE = 1.0 / math.sqrt(P)  # 1/sqrt(hd)


FULL = Cfg()


def build_program(cfg: Cfg):
    """Builds the per-core Bass/Tile program (SPMD: same NEFF on all cores)."""
    S, NK, H_LOC, CHUNK, NCH, TPC = cfg.S, cfg.NK, cfg.H_LOC, cfg.CHUNK, cfg.NCH, cfg.TPC
    DLOC = cfg.DLOC

    nc = bass.Bass("TRN2", target_bir_lowering=False, debug=False,
                   num_devices=cfg.n_cores)

    # ---- DRAM I/O ----
    xt_d = nc.dram_tensor("xt", [NCH, 2, P, (NK // 2) * CHUNK], BF16,
                          kind="ExternalInput").ap()
    wq_d = nc.dram_tensor("wq", [H_LOC, P, NK * P], BF16, kind="ExternalInput").ap()
    wk_d = nc.dram_tensor("wk", [H_LOC, P, NK * P], BF16, kind="ExternalInput").ap()
    wv_d = nc.dram_tensor("wv", [P, NK, DLOC], BF16, kind="ExternalInput").ap()
    wo_d = nc.dram_tensor("wo", [cfg.D // CHUNK, P, H_LOC * CHUNK], BF16,
                          kind="ExternalInput").ap()
    cos_d = nc.dram_tensor("cosS", [P, S], BF16, kind="ExternalInput").ap()
    sin_d = nc.dram_tensor("sinm", [P, S], BF16, kind="ExternalInput").ap()
    triu_d = nc.dram_tensor("triu", [P, P], BF16, kind="ExternalInput").ap()
    ones_d = nc.dram_tensor("ones128", [P, P], BF16, kind="ExternalInput").ap()
    swp_d = nc.dram_tensor("swap128", [P, P], BF16, kind="ExternalInput").ap()
    ot_d = nc.dram_tensor("ot", [S, cfg.D], BF16, kind="ExternalOutput").ap()

    with tile.TileContext(nc) as tc:
        with tc.tile_pool(name="const", bufs=1) as const_pool, \
             tc.tile_pool(name="persist", bufs=1) as persist, \
             tc.tile_pool(name="xtp", bufs=2) as xtp, \
             tc.tile_pool(name="wqk", bufs=2) as wqkp, \
             tc.tile_pool(name="qtp", bufs=H_LOC + 3) as qtp, \
             tc.tile_pool(name="rawp", bufs=4) as rawp, \
             tc.tile_pool(name="pp", bufs=4 * TPC + 4) as pp, \
             tc.tile_pool(name="atp", bufs=H_LOC + 2) as atp, \
             tc.tile_pool(name="recp", bufs=2) as recp, \
             tc.tile_pool(name="osbp", bufs=4) as osbp, \
             tc.tile_pool(name="psA", bufs=2, space="PSUM") as psA, \
             tc.tile_pool(name="psS", bufs=2, space="PSUM") as psS, \
             tc.tile_pool(name="psAT", bufs=3, space="PSUM") as psAT, \
             tc.tile_pool(name="psR", bufs=1, space="PSUM") as psR:

            # constants (gpsimd DMA queue; small transfers, emitted after the
            # first W pieces so they don't delay the first matmul)
            triu = const_pool.tile([P, P], BF16, name="triu")
            ones128 = const_pool.tile([P, P], BF16, name="ones128")
            swap128 = const_pool.tile([P, P], BF16, name="swap128")
            cosS = const_pool.tile([P, S], BF16, name="cosS")
            sinm = const_pool.tile([P, S], BF16, name="sinm")

            def emit_cos_sin():
                nc.gpsimd.dma_start(swap128, swp_d)
                for j in range(4):
                    sl = slice(j * (S // 4), (j + 1) * (S // 4))
                    nc.gpsimd.dma_start(cosS[:, sl], cos_d[:, sl])
                    nc.gpsimd.dma_start(sinm[:, sl], sin_d[:, sl])
                nc.gpsimd.dma_start(triu, triu_d)
                nc.gpsimd.dma_start(ones128, ones_d)

            # persistent tensors: K^T per head, natural V, Wv, Wo
            KT = []
            for h in range(H_LOC):
                kt_h = persist.tile([P, S], BF16, name=f"kt{h}", tag=f"kt{h}")
                KT.append(kt_h)
            Vn = persist.tile([P, S // P, DLOC], BF16, name="vnat", tag="vnat")
            wv_pers = persist.tile([P, NK, DLOC], BF16, name="wv_pers",
                                   tag="wv_pers")
            NGR = cfg.D // CHUNK
            wo_pers = persist.tile([P, NGR, H_LOC, CHUNK], BF16,
                                   name="wo_pers", tag="wo_pers")

            def emit_wv():
                for q in range(8):
                    ksl = slice(q * (NK // 8), (q + 1) * (NK // 8))
                    nc.gpsimd.dma_start(wv_pers[:, ksl, :], wv_d[:, ksl, :])

            NKH = NK // 2
            xts_all = {}     # ch -> [half0, half1]
            qt_all = {}      # (ch, h) -> qt tile
            at_all = {}      # (ch, h) -> at tile

            def alloc_xt(ch):
                xts = [xtp.tile([P, NKH, CHUNK], BF16,
                                name=f"xt_{ch}_{half}", tag="xt")
                       for half in range(2)]
                xts_all[ch] = xts

            def emit_xt_quarter(ch, q8, fine=False):
                """One of 8 quarter-DMAs for chunk ch's x^T strip."""
                half, q = divmod(q8, 4)
                xh = xts_all[ch][half]
                src = xt_d[ch, half].rearrange("p (k c) -> p k c", c=CHUNK)
                kq = NKH // 4
                if fine:
                    for j in range(kq):
                        ksl = slice(q * kq + j, q * kq + j + 1)
                        nc.sync.dma_start(xh[:, ksl, :], src[:, ksl, :])
                else:
                    ksl = slice(q * kq, (q + 1) * kq)
                    nc.sync.dma_start(xh[:, ksl, :], src[:, ksl, :])

            def emit_xt(ch, fine=False):
                alloc_xt(ch)
                for q8 in range(8):
                    emit_xt_quarter(ch, q8, fine=(fine and q8 == 0))

            def emit_qk_unit(ch, which, h, after_w_hook=None):
                """One q/k projection unit: W load + NK matmuls + RoPE."""
                s0 = ch * CHUNK
                xts = xts_all[ch]
                w_dram = {"q": wq_d, "k": wk_d}[which]
                wt = wqkp.tile([P, NK, P], BF16,
                               name=f"w{which}_{ch}_{h}", tag="wqk")
                wsrc = w_dram[h].rearrange("p (k m) -> p k m", m=P)
                npieces = (8 if which == "q" else 4) if (ch == 0 and h == 0) else 2
                for q in range(npieces):
                    ksl = slice(q * (NK // npieces), (q + 1) * (NK // npieces))
                    nc.gpsimd.dma_start(wt[:, ksl, :], wsrc[:, ksl, :])
                if after_w_hook is not None:
                    after_w_hook()
                ps = psA.tile([P, CHUNK], F32,
                              name=f"ps_{which}_{ch}_{h}", tag="psA")
                for k in range(NK):
                    nc.tensor.matmul(ps, wt[:, k, :],
                                     xts[k // NKH][:, k % NKH, :],
                                     start=(k == 0), stop=(k == NK - 1))
                raw = rawp.tile([P, CHUNK], BF16,
                                name=f"raw_{which}_{ch}_{h}", tag="raw")
                nc.vector.tensor_copy(raw, ps)
                # RoPE: rot = raw*C2 + swap(raw)*S2m
                ps2 = psR.tile([P, CHUNK], F32,
                               name=f"psw_{which}_{ch}_{h}", tag="psR")
                nc.tensor.matmul(ps2, swap128, raw, start=True, stop=True)
                if which == "q":
                    dst = qtp.tile([P, CHUNK], BF16,
                                   name=f"qt_{ch}_{h}", tag="qt")
                    qt_all[(ch, h)] = dst
                else:
                    dst = KT[h][:, s0:s0 + CHUNK]
                tmp2 = rawp.tile([P, CHUNK], BF16,
                                 name=f"tmp2_{which}_{ch}_{h}", tag="tmp2")
                nc.vector.tensor_mul(dst, raw, cosS[:, s0:s0 + CHUNK])
                nc.vector.tensor_mul(tmp2, ps2, sinm[:, s0:s0 + CHUNK])
                nc.vector.tensor_add(dst, dst, tmp2)

            def emit_v_unit(ch, tl):
                """V projection for one t-tile, all heads, directly in natural
                [t, hd] layout: stationary = x^T block, moving = Wv columns."""
                xts = xts_all[ch]
                ps = psA.tile([P, DLOC], F32, name=f"psv_{ch}_{tl}", tag="psA")
                tsl = slice(tl * P, (tl + 1) * P)
                for k in range(NK):
                    nc.tensor.matmul(ps, xts[k // NKH][:, k % NKH, tsl],
                                     wv_pers[:, k, :],
                                     start=(k == 0), stop=(k == NK - 1))
                # V units run in outproj windows where ScalarE is idle
                nc.scalar.copy(Vn[:, ch * TPC + tl, :], ps)

            def emit_attn_head(ch, h):
                """Attention for (chunk, head): transposed scores -> exp ->
                PV, rowsum via ones-stationary pass, fused normalize."""
                n_t = (ch + 1) * TPC
                qt_h = qt_all[(ch, h)]
                psat = psAT.tile([P, CHUNK], F32, name=f"psat_{ch}_{h}",
                                 tag="psAT")
                pts = []
                pending = None
                for tb in range(n_t):
                    s_lo = max(0, tb - ch * TPC) * P
                    pss = psS.tile([P, CHUNK], F32,
                                   name=f"pss_{ch}_{h}_{tb}", tag="psS")
                    nc.tensor.matmul(pss[:, s_lo:], KT[h][:, tb * P:(tb + 1) * P],
                                     qt_h[:, s_lo:], start=True, stop=True)
                    pt = pp.tile([P, CHUNK], BF16,
                                 name=f"pt_{ch}_{h}_{tb}", tag="pt")
                    nc.scalar.activation(pt[:, s_lo:], pss[:, s_lo:],
                                         mybir.ActivationFunctionType.Exp,
                                         scale=cfg.SCALE)
                    if tb >= ch * TPC:
                        nc.vector.tensor_mul(pt[:, s_lo:s_lo + P],
                                             pt[:, s_lo:s_lo + P], triu)
                    if pending is not None:
                        ptb, plo, ppt = pending
                        nc.tensor.matmul(psat[:, plo:],
                                         Vn[:, ptb, h * P:(h + 1) * P],
                                         ppt[:, plo:],
                                         start=(ptb == 0), stop=False)
                    pending = (tb, s_lo, pt)
                    pts.append((tb, s_lo, pt))
                ptb, plo, ppt = pending
                nc.tensor.matmul(psat[:, plo:], Vn[:, ptb, h * P:(h + 1) * P],
                                 ppt[:, plo:], start=(ptb == 0), stop=True)
                # rowsum over t (partition axis) via all-ones stationary:
                # every output partition receives sum_t P^T[t, s] -- i.e. the
                # rowsum arrives pre-broadcast.  Consecutive matmuls share the
                # ones stationary (deduped to one ldweights).
                rs = psR.tile([P, CHUNK], F32, name=f"rs_{ch}_{h}", tag="psR")
                for tb, s_lo, pt in pts:
                    nc.tensor.matmul(rs[:, s_lo:], ones128, pt[:, s_lo:],
                                     start=(tb == 0), stop=(tb == n_t - 1))
                # 1/rowsum as exp(-ln(rowsum)) on ScalarE: both functions live
                # in the natural_log_exp table set (no table switching), and
                # the DVE reciprocal at [128,512] would cost 8 cyc/element.
                lnrs = recp.tile([P, CHUNK], F32,
                                 name=f"lnrs_{ch}_{h}", tag="lnrs")
                nc.scalar.activation(lnrs, rs,
                                     mybir.ActivationFunctionType.Ln)
                recipb = recp.tile([P, CHUNK], F32,
                                   name=f"rec_{ch}_{h}", tag="rec")
                nc.scalar.activation(recipb, lnrs,
                                     mybir.ActivationFunctionType.Exp,
                                     scale=-1.0)
                at_h = atp.tile([P, CHUNK], BF16, name=f"at_{ch}_{h}", tag="at")
                nc.vector.tensor_mul(at_h, psat, recipb)
                at_all[(ch, h)] = at_h

            def emit_outproj(ch, filler_units):
                """Output projection for s-chunk ch, interleaved with the
                given list of zero-arg emit callbacks (projection units /
                prefetches) so the PE never starves."""
                s0 = ch * CHUNK
                at_cur = [at_all[(ch, h)] for h in range(H_LOC)]
                nu = len(filler_units)
                NBLK = (NGR + 3) // 4
                n_iters = TPC * NBLK
                for it in range(n_iters):
                    stl, blk = divmod(it, NBLK)
                    ngs = list(range(blk * 4, min(blk * 4 + 4, NGR)))
                    psos = []
                    for j in range(len(ngs)):
                        pool, tg = (psS, "psS") if j < 2 else (psAT, "psAT")
                        pso = pool.tile([P, CHUNK], F32,
                                        name=f"pso_{ch}_{stl}_{blk}_{j}", tag=tg)
                        psos.append(pso)
                    for h in range(H_LOC):
                        lhs = at_cur[h][:, stl * P:(stl + 1) * P]
                        for j, ng in enumerate(ngs):
                            nc.tensor.matmul(psos[j], lhs, wo_pers[:, ng, h, :],
                                             start=(h == 0),
                                             stop=(h == H_LOC - 1))
                    for j, ng in enumerate(ngs):
                        osb = osbp.tile([P, CHUNK], BF16,
                                        name=f"osb_{ch}_{stl}_{blk}_{j}", tag="osb")
                        if j % 2:
                            nc.scalar.copy(osb, psos[j])
                        else:
                            nc.vector.tensor_copy(osb, psos[j])
                        srow = s0 + stl * P
                        nc.sync.dma_start(
                            ot_d[srow:srow + P, ng * CHUNK:(ng + 1) * CHUNK], osb)
                    for u in range(it * nu // n_iters,
                                   (it + 1) * nu // n_iters):
                        filler_units[u]()

            # ---------------- emission schedule ----------------
            emit_xt(0, fine=True)
            first = [True]

            def _cos_hook():
                if first[0]:
                    emit_cos_sin()
                    first[0] = False

            # chunk 0 projections, all up front.  GpSimd DMA queue order:
            # consts, Wq0+cos/sin, Wk0, Wv (4MB), remaining W, Wo -- each
            # lands just before its first consumer.  x^T(1) prefetches on the
            # SP queue behind x^T(0).
            emit_qk_unit(0, "q", 0, after_w_hook=_cos_hook)
            emit_qk_unit(0, "k", 0)
            emit_wv()
            if NCH > 1:
                emit_xt(1)
            for tl in range(TPC):
                emit_v_unit(0, tl)
            for h in range(1, H_LOC):
                emit_qk_unit(0, "q", h)
                emit_qk_unit(0, "k", h)

            for ng in range(NGR):
                nc.gpsimd.dma_start(
                    wo_pers[:, ng],
                    wo_d[ng].rearrange("p (h c) -> p h c", c=CHUNK))

            for ch in range(NCH):
                nxt = ch + 1
                # ---- attention window: heads of ch, interleaved with q/k
                # units of chunk ch+1 for heads 0..1 and x^T prefetch ----
                if 2 <= nxt < NCH:
                    # x^T(ch+1) prefetch; its buffers (chunk ch-1's) are
                    # long free -- proj(ch) finished last window
                    emit_xt(nxt)
                for h in range(H_LOC):
                    emit_attn_head(ch, h)
                    if nxt < NCH and h < 2:
                        emit_qk_unit(nxt, "q", h)
                        emit_qk_unit(nxt, "k", h)
                # ---- output projection window: interleave v(ch+1) units and
                # the remaining q/k units of ch+1 ----
                fillers = []
                if nxt < NCH:
                    for tl in range(TPC):
                        fillers.append(lambda tl=tl: emit_v_unit(nxt, tl))
                    for h in range(2, H_LOC):
                        fillers.append(lambda h=h: emit_qk_unit(nxt, "q", h))
                        fillers.append(lambda h=h: emit_qk_unit(nxt, "k", h))
                emit_outproj(ch, fillers)

    dedup_ldweights(nc)
    split_excess_waits(nc)
    return nc


# ---------------- host-side data prep ----------------

def _tile_w(w_cols: np.ndarray, NK: int) -> np.ndarray:
    """[D, 128] per-head weight slice -> [128, NK*128] (k-part, k-outer*col)."""
    D = w_cols.shape[0]
    return np.ascontiguousarray(
        w_cols.reshape(NK, P, P).transpose(1, 0, 2).reshape(P, NK * P))


_ROPE_PERM = np.concatenate([np.arange(0, P, 2), np.arange(1, P, 2)])


def prepare_core_inputs(cfg: Cfg, core: int, x, wq, wk, wv, wo, cos, sin):
    """Builds the in_map (dict of numpy arrays) for one core."""
    bf = ml_dtypes.bfloat16
    S, D, H_LOC, CHUNK, NK, NCH = cfg.S, cfg.D, cfg.H_LOC, cfg.CHUNK, cfg.NK, cfg.NCH
    DLOC = cfg.DLOC
    c0 = core * DLOC

    out = {}
    # xt: [NCH, 2, 128, (NK//2)*CHUNK]
    xt = np.empty((NCH, 2, P, (NK // 2) * CHUNK), dtype=bf)
    xTb = x.T.astype(bf)  # [D, S]
    for ch in range(NCH):
        for half in range(2):
            blk = xTb[half * (D // 2):(half + 1) * (D // 2),
                      ch * CHUNK:(ch + 1) * CHUNK]          # [D/2, CHUNK]
            blk = blk.reshape(NK // 2, P, CHUNK).transpose(1, 0, 2)
            xt[ch, half] = blk.reshape(P, (NK // 2) * CHUNK)
    out["xt"] = xt

    for name, w in (("wq", wq), ("wk", wk)):
        wt = np.empty((H_LOC, P, NK * P), dtype=bf)
        for h in range(H_LOC):
            cols = w[:, c0 + h * P: c0 + (h + 1) * P][:, _ROPE_PERM]
            wt[h] = _tile_w(cols.astype(bf), NK)
        out[name] = wt

    # wv: [128, NK, DLOC]; wv_t[p, k, j] = wv[k*128+p, c0+j]
    wv_loc = wv[:, c0:c0 + DLOC].astype(bf)                  # [D, DLOC]
    out["wv"] = np.ascontiguousarray(
        wv_loc.reshape(NK, P, DLOC).transpose(1, 0, 2))

    # wo: [D//CHUNK, 128, H_LOC*CHUNK]; wo[ng, p, h*CHUNK+nl] = Wo[c0+h*128+p, ng*CHUNK+nl]
    wo_loc = wo[c0:c0 + DLOC, :].astype(bf)  # [DLOC, D]
    wo_t = np.empty((D // CHUNK, P, H_LOC * CHUNK), dtype=bf)
    for ng in range(D // CHUNK):
        blk = wo_loc[:, ng * CHUNK:(ng + 1) * CHUNK]     # [DLOC, CHUNK]
        blk = blk.reshape(H_LOC, P, CHUNK).transpose(1, 0, 2)
        wo_t[ng] = blk.reshape(P, H_LOC * CHUNK)
    out["wo"] = wo_t

    cosT = cos.T.astype(np.float32)    # [64, S]
    sinT = sin.T.astype(np.float32)
    out["cosS"] = np.concatenate([cosT, cosT], 0).astype(bf)
    out["sinm"] = np.concatenate([-sinT, sinT], 0).astype(bf)

    # triu (incl. diagonal) masks the diagonal 128-block of P^T [t, s]
    out["triu"] = np.triu(np.ones((P, P), np.float32)).astype(bf)
    out["ones128"] = np.ones((P, P), np.float32).astype(bf)
    sw = np.zeros((P, P), np.float32)
    sw[(np.arange(P) + 64) % P, np.arange(P)] = 1.0
    out["swap128"] = sw.astype(bf)
    return out


_PROGRAM_CACHE = {}


def get_program(cfg: Cfg):
    key = (cfg.S, cfg.D, cfg.H_LOC, cfg.CHUNK, cfg.n_cores)
    if key not in _PROGRAM_CACHE:
        _PROGRAM_CACHE[key] = build_program(cfg)
    return _PROGRAM_CACHE[key]


def run(cfg: Cfg, inputs: dict, trace: bool = False):
    """Run the sharded kernel; returns (list of per-core ot partials, results obj)."""
    install_ntff_hook_shim()
    enable_ldw_opt()
    x = np.asarray(inputs["x"], np.float32)
    wq = np.asarray(inputs["weight_q"], np.float32)
    wk = np.asarray(inputs["weight_k"], np.float32)
    wv = np.asarray(inputs["weight_v"], np.float32)
    wo = np.asarray(inputs["weight_o"], np.float32)
    cos = np.asarray(inputs["freqs_cos"], np.float32)
    sin = np.asarray(inputs["freqs_sin"], np.float32)

    nc = get_program(cfg)
    in_maps = [prepare_core_inputs(cfg, c, x, wq, wk, wv, wo, cos, sin)
               for c in range(cfg.n_cores)]
    res = bass_utils.run_bass_kernel_spmd(
        nc, in_maps, core_ids=list(range(cfg.n_cores)), trace=trace)
    return [r["ot"] for r in res.results], res


def kernel(**inputs) -> np.ndarray:
    ots, _ = run(FULL, inputs, trace=False)
    acc = np.zeros(ots[0].shape, dtype=np.float64)
    for ot in ots:
        acc += np.asarray(ot, dtype=np.float64)
    return np.ascontiguousarray(acc.astype(np.float32))


# revision 21
# speedup vs baseline: 1.0531x; 1.0103x over previous
"""Trainium2 Bass kernel for causal multi-head attention with RoPE
(nn_Attention: S=2048, D=4096, H=32, hd=128), tensor-parallel over heads
across 8 NeuronCores.

v2 strategy (per core, 4 heads):
  - Q^T/K^T projections head-major in [hd, S] layout (lhsT = W tile,
    rhs = x^T strip), bf16. RoPE via host-permuted [re;im] split:
    rot = raw*C2 + swap(raw)*S2m with a 128x128 swap matmul on the PE.
  - V projected DIRECTLY into natural [t, hd] layout: lhsT = x^T block
    [k,t-128] (stationary), rhs = Wv columns of all 4 heads [k, 512].
    No PE transposes for V; Wv is persistent in SBUF (loaded once).
  - Attention computes scores TRANSPOSED: scoresT[t, s-chunk] =
    (K^T tile)^T @ Q^T, so exp(scoresT) on ScalarE lands directly in the
    P^T layout that the PV matmul streams -- the per-block PE transposes
    of P from v1 are gone entirely.  Causal masking: t-tiles past the
    diagonal are skipped (ragged s_lo starts); the diagonal 128-block is
    masked multiplicatively (triu) on the DVE after exp.
  - Softmax denominators: rowsum over t is a partition-axis sum, done as
    one extra PE pass per head with an all-ones 128x128 stationary
    (ldweights deduped): out[p, s] = sum_t P^T[t, s] for every p, i.e.
    the rowsum is produced pre-broadcast across all partitions.  A
    single reciprocal_approx_fast (DVE) gives 1/rowsum, and the
    normalize is fused into the psum->sbuf copy of A^T (tensor_mul).
  - Output projection unchanged: O^T partial accumulated over the 4
    local heads, 4 concurrent psum groups sharing the stationary.
    Partials are written bf16; host sums the 8 partials in float64.

Scheduling: weight/const DMAs issue from the (otherwise idle) GpSimd
queue, x^T strips + outputs from SP.  ScalarE runs ONLY exp.  Emission
interleaves chunk ch's attention with chunk ch+1's q/k projections for
heads 0-1 and chunk ch's output projection with the remaining
projection units, so the PE stays fed through the Act-heavy late-chunk
attention windows.  x^T strips prefetch two windows ahead.
"""

import math
import sys
import types

import numpy as np
import ml_dtypes

import concourse.bass as bass
import concourse.tile as tile
import concourse.mybir as mybir
from concourse import bass_utils

BF16 = mybir.dt.bfloat16
F32 = mybir.dt.float32
P = 128


def enable_ldw_opt():
    """Flip walrus's --enable-ldw-opt to true (bass_utils hardcodes false).
    Patches run_command to rewrite the flag in the walrus argv."""
    import os
    if os.environ.get("BASS_LDW_OPT", "0") != "1":
        return
    if getattr(bass_utils, "_ldw_patch", False):
        return
    orig = bass_utils.run_command

    def patched(argv, **kwargs):
        argv = ["--enable-ldw-opt=true" if a == "--enable-ldw-opt=false" else a
                for a in argv]
        return orig(argv, **kwargs)

    bass_utils.run_command = patched
    bass_utils._ldw_patch = True


def install_ntff_hook_shim():
    """Make trace=True work under axon (antenv.axon_hooks is absent here)."""
    try:
        import antenv.axon_hooks  # noqa
        return
    except ImportError:
        pass
    try:
        import antenv
        from trn_agent_boot.trn_boot import _ntff_profile_via_ctypes
        hook = _ntff_profile_via_ctypes('/opt/axon/libaxon_pjrt.so')
        mod = types.ModuleType('antenv.axon_hooks')
        mod.get_axon_ntff_profile_hook = lambda: hook
        mod.set_axon_ntff_profile_hook = lambda h: None
        sys.modules['antenv.axon_hooks'] = mod
        antenv.axon_hooks = mod
    except Exception:
        pass


def dedup_ldweights(nc):
    """Remove an InstLdweights when the immediately preceding PE weight load
    has an identical stationary operand (consecutive matmuls sharing lhsT).
    Any waits on the removed load are transferred to the next instruction."""
    import concourse.mybir as _mb
    n = 0
    for f in nc.m.functions:
        for bb in f.blocks:
            new = []
            last_key = None
            pending_waits = []
            for inst in bb.instructions:
                ty = type(inst).__name__
                eng = getattr(inst, "engine", None)
                if eng == _mb.EngineType.PE:
                    if ty == "InstLdweights":
                        o = inst.ins[0]
                        key = (str(getattr(o, "memref", "")), o.offset,
                               str(o.ap), str(getattr(o, "dtype", "")),
                               getattr(inst, "is_transpose", None),
                               getattr(inst, "tile_position", None))
                        if key == last_key:
                            si = getattr(inst, "sync_info", None)
                            if si is not None and si.on_wait:
                                pending_waits.extend(si.on_wait)
                            n += 1
                            continue   # drop this load
                        last_key = key
                    elif ty in ("InstMatmult", "InstEventSemaphore", "InstNoOp"):
                        pass           # none of these clobber loaded weights
                    else:
                        last_key = None
                    if pending_waits:
                        si = getattr(inst, "sync_info", None)
                        if si is None:
                            inst.sync_info = _mb.SyncInfo(
                                on_wait=list(pending_waits), on_update=[])
                        else:
                            si.on_wait = list(pending_waits) + list(si.on_wait)
                        pending_waits = []
                new.append(inst)
            assert not pending_waits
            bb.instructions[:] = new
    return n


def split_excess_waits(nc, max_waits=1):
    """This walrus build accepts only one sync-wait per instruction; split
    extra waits into preceding wait-only NoOps on the same engine."""
    n = 0
    for f in nc.m.functions:
        for bb in f.blocks:
            new = []
            for inst in bb.instructions:
                si = getattr(inst, "sync_info", None)
                waits = list(si.on_wait) if (si is not None and si.on_wait) else []
                if len(waits) > max_waits:
                    extra, keep = waits[:-max_waits], waits[-max_waits:]
                    for j, w in enumerate(extra):
                        new.append(mybir.InstNoOp(
                            name=f"{inst.name}_sw{j}",
                            engine=inst.engine,
                            bass_nofuse=True,
                            sync_info=mybir.SyncInfo(on_wait=[w], on_update=[]),
                        ))
                    si.on_wait = keep
                    n += 1
                new.append(inst)
            bb.instructions[:] = new
    return n


class Cfg:
    def __init__(self, S=2048, D=4096, H_LOC=4, CHUNK=512, n_cores=8):
        self.S = S              # sequence length
        self.D = D              # model dim (= contraction dim of projections)
        self.H_LOC = H_LOC      # heads per core
        self.CHUNK = CHUNK      # s-chunk size (outer loop granularity)
        self.n_cores = n_cores
        self.NK = D // P        # k-tiles in projections
        self.NCH = S // CHUNK   # number of s-chunks
        self.TPC = CHUNK // P   # s/t tiles per chunk (must be 4 for 512)
        self.DLOC = H_LOC * P   # local head dims
        self.SCALE = 1.0 / math.sqrt(P)  # 1/sqrt(hd)


FULL = Cfg()


def build_program(cfg: Cfg):
    """Builds the per-core Bass/Tile program (SPMD: same NEFF on all cores)."""
    S, NK, H_LOC, CHUNK, NCH, TPC = cfg.S, cfg.NK, cfg.H_LOC, cfg.CHUNK, cfg.NCH, cfg.TPC
    DLOC = cfg.DLOC

    nc = bass.Bass("TRN2", target_bir_lowering=False, debug=False,
                   num_devices=cfg.n_cores)

    # ---- DRAM I/O ----
    xt_d = nc.dram_tensor("xt", [NCH, 2, P, (NK // 2) * CHUNK], BF16,
                          kind="ExternalInput").ap()
    wq_d = nc.dram_tensor("wq", [H_LOC, P, NK * P], BF16, kind="ExternalInput").ap()
    wk_d = nc.dram_tensor("wk", [H_LOC, P, NK * P], BF16, kind="ExternalInput").ap()
    wv_d = nc.dram_tensor("wv", [P, NK, DLOC], BF16, kind="ExternalInput").ap()
    wo_d = nc.dram_tensor("wo", [cfg.D // CHUNK, P, H_LOC * CHUNK], BF16,
                          kind="ExternalInput").ap()
    cos_d = nc.dram_tensor("cosS", [P, S], BF16, kind="ExternalInput").ap()
    sin_d = nc.dram_tensor("sinm", [P, S], BF16, kind="ExternalInput").ap()
    triu_d = nc.dram_tensor("triu", [P, P], BF16, kind="ExternalInput").ap()
    ones_d = nc.dram_tensor("ones128", [P, P], BF16, kind="ExternalInput").ap()
    swp_d = nc.dram_tensor("swap128", [P, P], BF16, kind="ExternalInput").ap()
    ot_d = nc.dram_tensor("ot", [S, cfg.D], BF16, kind="ExternalOutput").ap()

    with tile.TileContext(nc) as tc:
        with tc.tile_pool(name="const", bufs=1) as const_pool, \
             tc.tile_pool(name="persist", bufs=1) as persist, \
             tc.tile_pool(name="xtp", bufs=2) as xtp, \
             tc.tile_pool(name="wqk", bufs=2) as wqkp, \
             tc.tile_pool(name="qtp", bufs=H_LOC + 3) as qtp, \
             tc.tile_pool(name="rawp", bufs=4) as rawp, \
             tc.tile_pool(name="pp", bufs=4 * TPC + 4) as pp, \
             tc.tile_pool(name="atp", bufs=H_LOC + 2) as atp, \
             tc.tile_pool(name="recp", bufs=2) as recp, \
             tc.tile_pool(name="osbp", bufs=4) as osbp, \
             tc.tile_pool(name="psA", bufs=2, space="PSUM") as psA, \
             tc.tile_pool(name="psS", bufs=2, space="PSUM") as psS, \
             tc.tile_pool(name="psAT", bufs=3, space="PSUM") as psAT, \
             tc.tile_pool(name="psR", bufs=1, space="PSUM") as psR:

            # constants (gpsimd DMA queue; small transfers, emitted after the
            # first W pieces so they don't delay the first matmul)
            triu = const_pool.tile([P, P], BF16, name="triu")
            ones128 = const_pool.tile([P, P], BF16, name="ones128")
            swap128 = const_pool.tile([P, P], BF16, name="swap128")
            cosS = const_pool.tile([P, S], BF16, name="cosS")
            sinm = const_pool.tile([P, S], BF16, name="sinm")

            def emit_cos_sin():
                nc.gpsimd.dma_start(swap128, swp_d)
                for j in range(4):
                    sl = slice(j * (S // 4), (j + 1) * (S // 4))
                    nc.gpsimd.dma_start(cosS[:, sl], cos_d[:, sl])
                    nc.gpsimd.dma_start(sinm[:, sl], sin_d[:, sl])
                nc.gpsimd.dma_start(triu, triu_d)
                nc.gpsimd.dma_start(ones128, ones_d)

            # persistent tensors: K^T per head, natural V, Wv, Wo
            KT = []
            for h in range(H_LOC):
                kt_h = persist.tile([P, S], BF16, name=f"kt{h}", tag=f"kt{h}")
                KT.append(kt_h)
            Vn = persist.tile([P, S // P, DLOC], BF16, name="vnat", tag="vnat")
            wv_pers = persist.tile([P, NK, DLOC], BF16, name="wv_pers",
                                   tag="wv_pers")
            NGR = cfg.D // CHUNK
            wo_pers = persist.tile([P, NGR, H_LOC, CHUNK], BF16,
                                   name="wo_pers", tag="wo_pers")

            def emit_wv():
                for q in range(8):
                    ksl = slice(q * (NK // 8), (q + 1) * (NK // 8))
                    nc.gpsimd.dma_start(wv_pers[:, ksl, :], wv_d[:, ksl, :])

            NKH = NK // 2
            xts_all = {}     # ch -> [half0, half1]
            qt_all = {}      # (ch, h) -> qt tile
            at_all = {}      # (ch, h) -> at tile

            def alloc_xt(ch):
                xts = [xtp.tile([P, NKH, CHUNK], BF16,
                                name=f"xt_{ch}_{half}", tag="xt")
                       for half in range(2)]
                xts_all[ch] = xts

            def emit_xt_quarter(ch, q8, fine=False):
                """One of 8 quarter-DMAs for chunk ch's x^T strip."""
                half, q = divmod(q8, 4)
                xh = xts_all[ch][half]
                src = xt_d[ch, half].rearrange("p (k c) -> p k c", c=CHUNK)
                kq = NKH // 4
                if fine:
                    for j in range(kq):
                        ksl = slice(q * kq + j, q * kq + j + 1)
                        nc.sync.dma_start(xh[:, ksl, :], src[:, ksl, :])
                else:
                    ksl = slice(q * kq, (q + 1) * kq)
                    nc.sync.dma_start(xh[:, ksl, :], src[:, ksl, :])

            def emit_xt(ch, fine=False):
                alloc_xt(ch)
                for q8 in range(8):
                    emit_xt_quarter(ch, q8, fine=(fine and q8 == 0))

            def emit_qk_unit(ch, which, h, after_w_hook=None):
                """One q/k projection unit: W load + NK matmuls + RoPE."""
                s0 = ch * CHUNK
                xts = xts_all[ch]
                w_dram = {"q": wq_d, "k": wk_d}[which]
                wt = wqkp.tile([P, NK, P], BF16,
                               name=f"w{which}_{ch}_{h}", tag="wqk")
                wsrc = w_dram[h].rearrange("p (k m) -> p k m", m=P)
                npieces = (8 if which == "q" else 4) if (ch == 0 and h == 0) else 2
                for q in range(npieces):
                    ksl = slice(q * (NK // npieces), (q + 1) * (NK // npieces))
                    nc.gpsimd.dma_start(wt[:, ksl, :], wsrc[:, ksl, :])
                if after_w_hook is not None:
                    after_w_hook()
                ps = psA.tile([P, CHUNK], F32,
                              name=f"ps_{which}_{ch}_{h}", tag="psA")
                for k in range(NK):
                    nc.tensor.matmul(ps, wt[:, k, :],
                                     xts[k // NKH][:, k % NKH, :],
                                     start=(k == 0), stop=(k == NK - 1))
                raw = rawp.tile([P, CHUNK], BF16,
                                name=f"raw_{which}_{ch}_{h}", tag="raw")
                nc.vector.tensor_copy(raw, ps)
                # RoPE: rot = raw*C2 + swap(raw)*S2m
                ps2 = psR.tile([P, CHUNK], F32,
                               name=f"psw_{which}_{ch}_{h}", tag="psR")
                nc.tensor.matmul(ps2, swap128, raw, start=True, stop=True)
                if which == "q":
                    dst = qtp.tile([P, CHUNK], BF16,
                                   name=f"qt_{ch}_{h}", tag="qt")
                    qt_all[(ch, h)] = dst
                else:
                    dst = KT[h][:, s0:s0 + CHUNK]
                tmp2 = rawp.tile([P, CHUNK], BF16,
                                 name=f"tmp2_{which}_{ch}_{h}", tag="tmp2")
                nc.vector.tensor_mul(dst, raw, cosS[:, s0:s0 + CHUNK])
                nc.vector.tensor_mul(tmp2, ps2, sinm[:, s0:s0 + CHUNK])
                nc.vector.tensor_add(dst, dst, tmp2)

            def emit_v_unit(ch, tl):
                """V projection for one t-tile, all heads, directly in natural
                [t, hd] layout: stationary = x^T block, moving = Wv columns."""
                xts = xts_all[ch]
                ps = psA.tile([P, DLOC], F32, name=f"psv_{ch}_{tl}", tag="psA")
                tsl = slice(tl * P, (tl + 1) * P)
                for k in range(NK):
                    nc.tensor.matmul(ps, xts[k // NKH][:, k % NKH, tsl],
                                     wv_pers[:, k, :],
                                     start=(k == 0), stop=(k == NK - 1))
                # V units run in outproj windows where ScalarE is idle
                nc.scalar.copy(Vn[:, ch * TPC + tl, :], ps)

            def emit_attn_head(ch, h):
                """Attention for (chunk, head): transposed scores -> exp ->
                PV, rowsum via ones-stationary pass, fused normalize."""
                n_t = (ch + 1) * TPC
                qt_h = qt_all[(ch, h)]
                psat = psAT.tile([P, CHUNK], F32, name=f"psat_{ch}_{h}",
                                 tag="psAT")
                pts = []
                pending = None
                for tb in range(n_t):
                    s_lo = max(0, tb - ch * TPC) * P
                    pss = psS.tile([P, CHUNK], F32,
                                   name=f"pss_{ch}_{h}_{tb}", tag="psS")
                    nc.tensor.matmul(pss[:, s_lo:], KT[h][:, tb * P:(tb + 1) * P],
                                     qt_h[:, s_lo:], start=True, stop=True)
                    pt = pp.tile([P, CHUNK], BF16,
                                 name=f"pt_{ch}_{h}_{tb}", tag="pt")
                    nc.scalar.activation(pt[:, s_lo:], pss[:, s_lo:],
                                         mybir.ActivationFunctionType.Exp,
                                         scale=cfg.SCALE)
                    if tb >= ch * TPC:
                        nc.vector.tensor_mul(pt[:, s_lo:s_lo + P],
                                             pt[:, s_lo:s_lo + P], triu)
                    if pending is not None:
                        ptb, plo, ppt = pending
                        nc.tensor.matmul(psat[:, plo:],
                                         Vn[:, ptb, h * P:(h + 1) * P],
                                         ppt[:, plo:],
                                         start=(ptb == 0), stop=False)
                    pending = (tb, s_lo, pt)
                    pts.append((tb, s_lo, pt))
                ptb, plo, ppt = pending
                nc.tensor.matmul(psat[:, plo:], Vn[:, ptb, h * P:(h + 1) * P],
                                 ppt[:, plo:], start=(ptb == 0), stop=True)
                # rowsum over t (partition axis) via all-ones stationary:
                # every output partition receives sum_t P^T[t, s] -- i.e. the
                # rowsum arrives pre-broadcast.  Consecutive matmuls share the
                # ones stationary (deduped to one ldweights).
                rs = psR.tile([P, CHUNK], F32, name=f"rs_{ch}_{h}", tag="psR")
                for tb, s_lo, pt in pts:
                    nc.tensor.matmul(rs[:, s_lo:], ones128, pt[:, s_lo:],
                                     start=(tb == 0), stop=(tb == n_t - 1))
                # 1/rowsum as exp(-ln(rowsum)) on ScalarE: both functions live
                # in the natural_log_exp table set (no table switching), and
                # the DVE reciprocal at [128,512] would cost 8 cyc/element.
                lnrs = recp.tile([P, CHUNK], F32,
                                 name=f"lnrs_{ch}_{h}", tag="lnrs")
                nc.scalar.activation(lnrs, rs,
                                     mybir.ActivationFunctionType.Ln)
                recipb = recp.tile([P, CHUNK], F32,
                                   name=f"rec_{ch}_{h}", tag="rec")
                nc.scalar.activation(recipb, lnrs,
                                     mybir.ActivationFunctionType.Exp,
                                     scale=-1.0)
                at_h = atp.tile([P, CHUNK], BF16, name=f"at_{ch}_{h}", tag="at")
                nc.vector.tensor_mul(at_h, psat, recipb)
                at_all[(ch, h)] = at_h

            def emit_outproj(ch, filler_units, it_lo=0, it_hi=None,
                             use_psA=False):
                """Output projection for s-chunk ch (iteration subrange),
                interleaved with the given list of zero-arg emit callbacks
                (projection units / prefetches) so the PE never starves.
                use_psA draws the psum groups from psA (2 groups of 2 ngs)
                for ranges emitted inside attention windows."""
                s0 = ch * CHUNK
                at_cur = [at_all[(ch, h)] for h in range(H_LOC)]
                nu = len(filler_units)
                NBLK = (NGR + 3) // 4
                n_iters = TPC * NBLK
                if it_hi is None:
                    it_hi = n_iters
                for it in range(it_lo, it_hi):
                    stl, blk = divmod(it, NBLK)
                    ngs = list(range(blk * 4, min(blk * 4 + 4, NGR)))
                    if use_psA:
                        ngs = ngs[:2] + ngs[2:]  # emitted as two sub-groups
                    psos = []
                    for j in range(len(ngs)):
                        if use_psA:
                            pool, tg = psA, "psA"
                        else:
                            pool, tg = (psS, "psS") if j < 2 else (psAT, "psAT")
                        pso = pool.tile([P, CHUNK], F32,
                                        name=f"pso_{ch}_{stl}_{blk}_{j}", tag=tg)
                        psos.append(pso)
                    for h in range(H_LOC):
                        lhs = at_cur[h][:, stl * P:(stl + 1) * P]
                        for j, ng in enumerate(ngs):
                            nc.tensor.matmul(psos[j], lhs, wo_pers[:, ng, h, :],
                                             start=(h == 0),
                                             stop=(h == H_LOC - 1))
                    for j, ng in enumerate(ngs):
                        osb = osbp.tile([P, CHUNK], BF16,
                                        name=f"osb_{ch}_{stl}_{blk}_{j}", tag="osb")
                        if j % 2:
                            nc.scalar.copy(osb, psos[j])
                        else:
                            nc.vector.tensor_copy(osb, psos[j])
                        srow = s0 + stl * P
                        nc.sync.dma_start(
                            ot_d[srow:srow + P, ng * CHUNK:(ng + 1) * CHUNK], osb)
                    k0 = (it - it_lo) * nu // (it_hi - it_lo)
                    k1 = (it - it_lo + 1) * nu // (it_hi - it_lo)
                    for u in range(k0, k1):
                        filler_units[u]()

            # ---------------- emission schedule ----------------
            emit_xt(0, fine=True)
            first = [True]

            def _cos_hook():
                if first[0]:
                    emit_cos_sin()
                    first[0] = False

            # chunk 0 projections, all up front.  GpSimd DMA queue order:
            # consts, Wq0+cos/sin, Wk0, Wv (4MB), remaining W, Wo -- each
            # lands just before its first consumer.  x^T(1) prefetches on the
            # SP queue behind x^T(0).
            emit_qk_unit(0, "q", 0, after_w_hook=_cos_hook)
            emit_qk_unit(0, "k", 0)
            emit_wv()
            if NCH > 1:
                emit_xt(1)
            for tl in range(TPC):
                emit_v_unit(0, tl)
            for h in range(1, H_LOC):
                emit_qk_unit(0, "q", h)
                emit_qk_unit(0, "k", h)

            for ng in range(NGR):
                nc.gpsimd.dma_start(
                    wo_pers[:, ng],
                    wo_d[ng].rearrange("p (h c) -> p h c", c=CHUNK))

            NBLK0 = (NGR + 3) // 4
            N_IT = TPC * NBLK0
            for ch in range(NCH):
                nxt = ch + 1
                last = (nxt == NCH)
                # ---- attention window: heads of ch, interleaved with q/k
                # units of chunk ch+1 for heads 0..1 and x^T prefetch; for
                # the (Act-bound, filler-less) last chunk, the tail of
                # outproj(ch-1) moves in here on the idle psA banks ----
                if 2 <= nxt < NCH:
                    # x^T(ch+1) prefetch; its buffers (chunk ch-1's) are
                    # long free -- proj(ch) finished last window
                    emit_xt(nxt)
                for h in range(H_LOC):
                    emit_attn_head(ch, h)
                    if not last and h < 2:
                        emit_qk_unit(nxt, "q", h)
                        emit_qk_unit(nxt, "k", h)
                    if last and ch >= 1 and h in (1, 2):
                        emit_outproj(ch - 1, [], it_lo=N_IT - 2 + (h - 1),
                                     it_hi=N_IT - 1 + (h - 1), use_psA=True)
                # ---- output projection window: interleave v(ch+1) units and
                # the remaining q/k units of ch+1 ----
                fillers = []
                if not last:
                    for tl in range(TPC):
                        fillers.append(lambda tl=tl: emit_v_unit(nxt, tl))
                    for h in range(2, H_LOC):
                        fillers.append(lambda h=h: emit_qk_unit(nxt, "q", h))
                        fillers.append(lambda h=h: emit_qk_unit(nxt, "k", h))
                emit_outproj(ch, fillers,
                             it_hi=(N_IT - 2) if nxt == NCH - 1 else None)

    dedup_ldweights(nc)
    split_excess_waits(nc)
    return nc


# ---------------- host-side data prep ----------------

def _tile_w(w_cols: np.ndarray, NK: int) -> np.ndarray:
    """[D, 128] per-head weight slice -> [128, NK*128] (k-part, k-outer*col)."""
    D = w_cols.shape[0]
    return np.ascontiguousarray(
        w_cols.reshape(NK, P, P).transpose(1, 0, 2).reshape(P, NK * P))


_ROPE_PERM = np.concatenate([np.arange(0, P, 2), np.arange(1, P, 2)])


def prepare_core_inputs(cfg: Cfg, core: int, x, wq, wk, wv, wo, cos, sin):
    """Builds the in_map (dict of numpy arrays) for one core."""
    bf = ml_dtypes.bfloat16
    S, D, H_LOC, CHUNK, NK, NCH = cfg.S, cfg.D, cfg.H_LOC, cfg.CHUNK, cfg.NK, cfg.NCH
    DLOC = cfg.DLOC
    c0 = core * DLOC

    out = {}
    # xt: [NCH, 2, 128, (NK//2)*CHUNK]
    xt = np.empty((NCH, 2, P, (NK // 2) * CHUNK), dtype=bf)
    xTb = x.T.astype(bf)  # [D, S]
    for ch in range(NCH):
        for half in range(2):
            blk = xTb[half * (D // 2):(half + 1) * (D // 2),
                      ch * CHUNK:(ch + 1) * CHUNK]          # [D/2, CHUNK]
            blk = blk.reshape(NK // 2, P, CHUNK).transpose(1, 0, 2)
            xt[ch, half] = blk.reshape(P, (NK // 2) * CHUNK)
    out["xt"] = xt

    for name, w in (("wq", wq), ("wk", wk)):
        wt = np.empty((H_LOC, P, NK * P), dtype=bf)
        for h in range(H_LOC):
            cols = w[:, c0 + h * P: c0 + (h + 1) * P][:, _ROPE_PERM]
            wt[h] = _tile_w(cols.astype(bf), NK)
        out[name] = wt

    # wv: [128, NK, DLOC]; wv_t[p, k, j] = wv[k*128+p, c0+j]
    wv_loc = wv[:, c0:c0 + DLOC].astype(bf)                  # [D, DLOC]
    out["wv"] = np.ascontiguousarray(
        wv_loc.reshape(NK, P, DLOC).transpose(1, 0, 2))

    # wo: [D//CHUNK, 128, H_LOC*CHUNK]; wo[ng, p, h*CHUNK+nl] = Wo[c0+h*128+p, ng*CHUNK+nl]
    wo_loc = wo[c0:c0 + DLOC, :].astype(bf)  # [DLOC, D]
    wo_t = np.empty((D // CHUNK, P, H_LOC * CHUNK), dtype=bf)
    for ng in range(D // CHUNK):
        blk = wo_loc[:, ng * CHUNK:(ng + 1) * CHUNK]     # [DLOC, CHUNK]
        blk = blk.reshape(H_LOC, P, CHUNK).transpose(1, 0, 2)
        wo_t[ng] = blk.reshape(P, H_LOC * CHUNK)
    out["wo"] = wo_t

    cosT = cos.T.astype(np.float32)    # [64, S]
    sinT = sin.T.astype(np.float32)
    out["cosS"] = np.concatenate([cosT, cosT], 0).astype(bf)
    out["sinm"] = np.concatenate([-sinT, sinT], 0).astype(bf)

    # triu (incl. diagonal) masks the diagonal 128-block of P^T [t, s]
    out["triu"] = np.triu(np.ones((P, P), np.float32)).astype(bf)
    out["ones128"] = np.ones((P, P), np.float32).astype(bf)
    sw = np.zeros((P, P), np.float32)
    sw[(np.arange(P) + 64) % P, np.arange(P)] = 1.0
    out["swap128"] = sw.astype(bf)
    return out


_PROGRAM_CACHE = {}


def get_program(cfg: Cfg):
    key = (cfg.S, cfg.D, cfg.H_LOC, cfg.CHUNK, cfg.n_cores)
    if key not in _PROGRAM_CACHE:
        _PROGRAM_CACHE[key] = build_program(cfg)
    return _PROGRAM_CACHE[key]


def run(cfg: Cfg, inputs: dict, trace: bool = False):
    """Run the sharded kernel; returns (list of per-core ot partials, results obj)."""
    install_ntff_hook_shim()
    enable_ldw_opt()
    x = np.asarray(inputs["x"], np.float32)
    wq = np.asarray(inputs["weight_q"], np.float32)
    wk = np.asarray(inputs["weight_k"], np.float32)
    wv = np.asarray(inputs["weight_v"], np.float32)
    wo = np.asarray(inputs["weight_o"], np.float32)
    cos = np.asarray(inputs["freqs_cos"], np.float32)
    sin = np.asarray(inputs["freqs_sin"], np.float32)

    nc = get_program(cfg)
    in_maps = [prepare_core_inputs(cfg, c, x, wq, wk, wv, wo, cos, sin)
               for c in range(cfg.n_cores)]
    res = bass_utils.run_bass_kernel_spmd(
        nc, in_maps, core_ids=list(range(cfg.n_cores)), trace=trace)
    return [r["ot"] for r in res.results], res


def kernel(**inputs) -> np.ndarray:
    ots, _ = run(FULL, inputs, trace=False)
    acc = np.zeros(ots[0].shape, dtype=np.float64)
    for ot in ots:
        acc += np.asarray(ot, dtype=np.float64)
    return np.ascontiguousarray(acc.astype(np.float32))
